# revision 58
# baseline (speedup 1.0000x reference)
"""Trainium2 Bass kernel for MultiHeadedAttention with learned per-key-position scaling.

Sharding over 8 NeuronCores: batch(2) x q-half(2) x head-half(2), fully
independent SPMD (no collectives): each core streams the FULL query in fp8
hi/lo form and computes the per-key-position divisor delta locally.

Key techniques vs the v1 baseline:
  - Q/K projections run as fp8e4 DoubleRow matmuls on a hi+lo split of the
    inputs (3-term product W*x ~ Wh*xh + Wl*xh + Wh*xl): 0.75x PE cost and
    half the input DMA of f32; the PSUM accumulation stays f32-exact enough
    (~2e-4) that scores still use the f32r path.
  - V projection runs single-term fp8 DoubleRow (0.25x PE cost).
  - Attention probabilities stay bf16 (the softmax has no max-subtraction,
    so exp() values exceed fp8 range); a subset of mask multiplies is
    routed to GPSIMD/Pool to offload the DVE.
  - The softmax denominator broadcast uses gpsimd partition_broadcast,
    issued at head end with the normalize multiply deferred into the next
    head's first pair so its latency fully hides.

Host combines per-core partial outputs (sum over head-halves + bo).
"""

import sys

for _p in ("/opt/trn_rl_repo",):
    if _p not in sys.path:
        sys.path.insert(0, _p)

import numpy as np
import ml_dtypes

BF16 = ml_dtypes.bfloat16
F8 = ml_dtypes.float8_e4m3

B, S, D, H, DK = 2, 2048, 768, 12, 64
NCORES = 8
SQ = S // 2          # query rows per core
HH = H // 2          # heads per core
DH = HH * DK         # 384 head dims per core

# tunables
BW = 512             # stream block width (projection inputs)
POOL_KCPS = ()           # kc-pairs whose mask-mult runs on gpsimd (per head);
                         # empty: the ~4us gpsimd op latency stalls the
                         # in-order PE queue more than it saves on DVE
WSCALE = 32.0        # fp8 weight pre-scale (avoids e4m3 subnormal underflow)

_cache = {}


def _build(s=S, sq=SQ, hh=HH, d=D, dk=DK, n_qh=2):
    import concourse.bass as bass
    import concourse.mybir as mybir
    import concourse.tile as tile
    from concourse import bacc

    f32 = mybir.dt.float32
    f32r = mybir.dt.float32r
    bf = mybir.dt.bfloat16
    f8 = mybir.dt.float8e4
    DR = mybir.MatmulPerfMode.DoubleRow
    Exp = mybir.ActivationFunctionType.Exp
    mult = mybir.AluOpType.mult
    add = mybir.AluOpType.add
    amin = mybir.AluOpType.min
    amax = mybir.AluOpType.max

    dh = hh * dk
    KC = s // 128        # key-position chunks (16)
    KCP = KC // 2        # kc pairs (8)
    C6 = d // 128        # d_model chunks (6)
    C3 = dh // 128       # output-dim chunks per core (3)
    NQ = sq // 512       # 512-wide q column blocks (attention) (2)
    QC = sq // 128       # q row chunks for output projection (8)
    NBS = s // BW        # full-seq stream blocks (4)
    KCL = BW // 128      # kc chunks per stream block (4)

    nc = bacc.Bacc("TRN2", target_bir_lowering=False, debug=False, num_devices=NCORES)

    t = {}
    # hi/lo fp8 streams, full sequence; layouts match SBUF tiles exactly
    t["qhl"] = nc.dram_tensor("qhl", [128, C6, 2, s], f8, kind="ExternalInput").ap()
    t["khl"] = nc.dram_tensor("khl", [128, C6, 2, s], f8, kind="ExternalInput").ap()
    t["v8"] = nc.dram_tensor("v8", [128, C6, 2, s], f8, kind="ExternalInput").ap()
    t["maskT"] = nc.dram_tensor("maskT", [s, sq], bf, kind="ExternalInput").ap()
    t["wq8"] = nc.dram_tensor("wq8", [128, C6, 2, dh], f8, kind="ExternalInput").ap()
    t["wk8"] = nc.dram_tensor("wk8", [128, C6, 2, dh], f8, kind="ExternalInput").ap()
    t["wv8"] = nc.dram_tensor("wv8", [128, C6, 2, dh], f8, kind="ExternalInput").ap()
    t["wo"] = nc.dram_tensor("wo", [dh, d], bf, kind="ExternalInput").ap()
    # wd8 columns: [wdh, wdh, wdl, 0] so both delta DR matmuls stride cleanly
    t["wd8"] = nc.dram_tensor("wd8", [128, C6, 4], f8, kind="ExternalInput").ap()
    t["bq"] = nc.dram_tensor("bq", [dh], f32, kind="ExternalInput").ap()
    t["bk"] = nc.dram_tensor("bk", [dh], f32, kind="ExternalInput").ap()
    t["bv"] = nc.dram_tensor("bv", [dh], f32, kind="ExternalInput").ap()
    t["bd"] = nc.dram_tensor("bd", [1], f32, kind="ExternalInput").ap()
    t["yp"] = nc.dram_tensor("yp", [sq, d], bf, kind="ExternalOutput").ap()

    def bcast(ap, n):
        # broadcast a 1-D DRAM vector across n partitions
        return bass.AP(tensor=ap.tensor, offset=ap.offset, ap=[[0, n]] + list(ap.ap))

    def rep0(ap):
        # stride-0 doubling of a singleton middle dim: [p, 1, n] -> [p, 2(0), n]
        naps = [list(dd) for dd in ap.ap]
        assert naps[1][1] == 1
        naps[1] = [0, 2]
        return bass.AP(tensor=ap.tensor, offset=ap.offset, ap=naps)

    with tile.TileContext(nc) as tc:
        with (
            tc.tile_pool(name="persist", bufs=1) as P,
            tc.tile_pool(name="pj", bufs=3, space="PSUM") as PJ,
            tc.tile_pool(name="xpp", bufs=1, space="PSUM") as XPP,
            tc.tile_pool(name="work", bufs=3) as W,
            tc.tile_pool(name="work2", bufs=4) as W2,
            tc.tile_pool(name="load", bufs=4) as L,
            tc.tile_pool(name="loadv", bufs=2) as LV,
        ):
            maskT = P.tile([128, KC, sq], bf)
            vsb = P.tile([128, KCP, 2, hh, dk + 1], bf)
            qTh = P.tile([128, C3, sq], f32r)    # head pairs packed on partitions
            kTh = P.tile([128, C3, s], f32r)
            xT = P.tile([128, C3, sq], bf)
            wq_sb = P.tile([128, C6, 2, dh], f8)
            wk_sb = P.tile([128, C6, 2, dh], f8)
            wv_sb = P.tile([128, C6, 2, dh], f8)
            wo_sb = P.tile([128, C3, d], bf)
            wd_sb = P.tile([128, C6, 4], f8)
            bqc = P.tile([128, C3], f32)
            bkc = P.tile([128, C3], f32)
            bvb = P.tile([128, hh, dk], f32)
            bdb = P.tile([128, 1], f32)
            # per-stream-block reciprocal-delta tiles: written as each q block
            # lands so attention's exp never waits on the full q stream
            rdts = [P.tile([128, KCL], f32, name=f"rdt{i}") for i in range(NBS)]

            # warm the ACT exp table while DMAs stream
            dummy = W2.tile([1, 2], f32, tag="dummy", bufs=1)
            nc.vector.memset(dummy, 0.0)
            nc.scalar.activation(dummy, dummy, Exp, scale=1.0)

            nc.sync.dma_start(wd_sb, t["wd8"])
            nc.gpsimd.dma_start(bdb, bcast(t["bd"], 128))
            nc.sync.dma_start(bqc, t["bq"].rearrange("(c p) -> p c", p=128))
            nc.vector.memset(vsb[:, :, :, :, dk : dk + 1], 1.0)

            # q-half ordering is handled host-side: the own half occupies
            # stream blocks [0, sq//BW); the other half [sq//BW, NBS).  The
            # key axis (khl/v8/maskT rows) is permuted identically host-side
            # so delta columns line up with key chunks.
            NBQ = sq // BW  # own-half q blocks (2)

            # --- stream DMA helpers (issued in priority order below) ---
            ktiles, vtiles = {}, {}

            def k_dma(blk):
                kb = L.tile([128, C6, 2, BW], f8, tag="ldk", bufs=3)
                nc.sync.dma_start(kb, t["khl"][:, :, :, blk * BW : (blk + 1) * BW])
                ktiles[blk] = kb

            def v_dma(blk):
                vb = LV.tile([128, C6, 2, BW], f8, tag="ldv")
                nc.sync.dma_start(vb, t["v8"][:, :, :, blk * BW : (blk + 1) * BW])
                vtiles[blk] = vb

            def mask_g(g):
                nc.sync.dma_start(
                    maskT[:, g * (KC // 4) : (g + 1) * (KC // 4), :],
                    t["maskT"].rearrange("(kc p) q -> p kc q", p=128)[
                        :, g * (KC // 4) : (g + 1) * (KC // 4), :
                    ],
                )

            # --- Q stream: projection (own half) + delta partials (full S).
            # DMA priority: everything attention pairs 0-3 need lands first;
            # other-half q blocks (delta only) stream behind k1/v1.
            qtiles = []
            for blk in range(NBS):
                qb = L.tile([128, C6, 2, BW], f8, tag="ldq")
                nc.sync.dma_start(qb, t["qhl"][:, :, :, blk * BW : (blk + 1) * BW])
                qtiles.append(qb)
                if blk == 0:
                    nc.sync.dma_start(wq_sb, t["wq8"])
                    nc.sync.dma_start(wk_sb, t["wk8"])
                    nc.sync.dma_start(bkc, t["bk"].rearrange("(c p) -> p c", p=128))
                elif blk == 1:
                    k_dma(0)
                    mask_g(0)
                    v_dma(0)
                    k_dma(1)
                    mask_g(1)
                    v_dma(1)
            nc.sync.dma_start(wv_sb, t["wv8"])
            nc.gpsimd.dma_start(
                bvb, bcast(t["bv"].rearrange("(h e) -> h e", h=hh), 128)
            )

            def q_delta(blk):
                # delta partials: z = (qh+ql)@(wdh+wdl)  (per kc column)
                qb = qtiles[blk]
                dps = PJ.tile([128, KCL], f32, tag="pj", name=f"dps{blk}")
                for kcl in range(KCL):
                    for c in range(C6):
                        lhs_pair = qb[:, c, :, kcl * 128 : (kcl + 1) * 128]
                        nc.tensor.matmul(
                            dps[:, kcl : kcl + 1],
                            lhsT=lhs_pair,
                            rhs=wd_sb[:, c, 0:2].rearrange("p (i o) -> p i o", o=1),
                            start=(c == 0),
                            stop=False,
                            perf_mode=DR,
                        )
                        nc.tensor.matmul(
                            dps[:, kcl : kcl + 1],
                            lhsT=lhs_pair,
                            rhs=wd_sb[:, c, 2:4].rearrange("p (i o) -> p i o", o=1),
                            start=False,
                            stop=(c == C6 - 1),
                            perf_mode=DR,
                        )
                # this block's reciprocal delta (host bdb = WSCALE*(bd+1))
                dloc = W2.tile([128, KCL], f32, tag="dloc", bufs=1, name=f"dl{blk}")
                nc.vector.tensor_scalar(
                    out=dloc, in0=dps, scalar1=bdb, scalar2=1.0 / WSCALE,
                    op0=add, op1=mult,
                )
                nc.vector.tensor_scalar(
                    out=dloc, in0=dloc, scalar1=1.0, scalar2=9.0, op0=amax, op1=amin
                )
                nc.vector.reciprocal(rdts[blk], dloc)

            for blk in range(NBQ):
                qb = qtiles[blk]
                q_delta(blk)
                if True:
                    for m in range(C3):
                        qp = PJ.tile([128, BW], f32, tag="pj")
                        first = True
                        for c in range(C6):
                            nc.tensor.matmul(
                                qp,
                                lhsT=wq_sb[:, c, :, m * 128 : (m + 1) * 128],
                                rhs=rep0(qb[:, c, 0:1, :]),
                                start=first,
                                stop=False,
                                perf_mode=DR,
                            )
                            first = False
                        for c in range(0, C6, 2):
                            nc.tensor.matmul(
                                qp,
                                lhsT=wq_sb[:, c : c + 2, 0, m * 128 : (m + 1) * 128],
                                rhs=qb[:, c : c + 2, 1, :],
                                start=False,
                                stop=(c == C6 - 2),
                                perf_mode=DR,
                            )
                        nc.vector.tensor_scalar(
                            out=qTh[:, m, blk * BW : (blk + 1) * BW],
                            in0=qp,
                            scalar1=bqc[:, m : m + 1],
                            scalar2=1.0 / WSCALE,
                            op0=add,
                            op1=mult,
                        )

            # --- stream emitters, interleaved with head-0 attention below ---
            def k_block(blk):
                kb = ktiles.pop(blk)
                for m in range(C3):
                    kp = PJ.tile([128, BW], f32, tag="pj")
                    first = True
                    for c in range(C6):
                        nc.tensor.matmul(
                            kp,
                            lhsT=wk_sb[:, c, :, m * 128 : (m + 1) * 128],
                            rhs=rep0(kb[:, c, 0:1, :]),
                            start=first,
                            stop=False,
                            perf_mode=DR,
                        )
                        first = False
                    for c in range(0, C6, 2):
                        nc.tensor.matmul(
                            kp,
                            lhsT=wk_sb[:, c : c + 2, 0, m * 128 : (m + 1) * 128],
                            rhs=kb[:, c : c + 2, 1, :],
                            start=False,
                            stop=(c == C6 - 2),
                            perf_mode=DR,
                        )
                    nc.vector.tensor_scalar(
                        out=kTh[:, m, blk * BW : (blk + 1) * BW],
                        in0=kp,
                        scalar1=bkc[:, m : m + 1],
                        scalar2=1.0 / WSCALE,
                        op0=add,
                        op1=mult,
                    )

            def v_block(blk):
                vb = vtiles.pop(blk)
                for kcl in range(KCL):
                    kc = blk * KCL + kcl
                    kcp, ip = kc // 2, kc % 2
                    vp = PJ.tile([128, dh], f32, tag="pj")
                    # 3-term: (vh,vl)@(wvh,wvh) per chunk + (vh_c,vh_c+1)@(wvl_c,wvl_c+1)
                    for c in range(C6):
                        nc.tensor.matmul(
                            vp,
                            lhsT=vb[:, c, :, kcl * 128 : (kcl + 1) * 128],
                            rhs=rep0(wv_sb[:, c, 0:1, :]),
                            start=(c == 0),
                            stop=False,
                            perf_mode=DR,
                        )
                    for c in range(0, C6, 2):
                        nc.tensor.matmul(
                            vp,
                            lhsT=vb[:, c : c + 2, 0, kcl * 128 : (kcl + 1) * 128],
                            rhs=wv_sb[:, c : c + 2, 1, :],
                            start=False,
                            stop=(c == C6 - 2),
                            perf_mode=DR,
                        )
                    nc.vector.scalar_tensor_tensor(
                        out=vsb[:, kcp, ip, :, 0:dk],
                        in0=vp.rearrange("p (h e) -> p h e", h=hh),
                        scalar=1.0 / WSCALE,
                        in1=bvb,
                        op0=mult,
                        op1=add,
                    )

            # prologue: blocks 0/1 DMAs were issued with the q stream above
            k_block(0)
            v_block(0)

            # --- attention: 6 heads x 8 kc-pairs ---
            pend = None
            for h in range(hh):
                hoff = (h % 2) * 64
                xps = XPP.tile([dk + 1, sq], f32, tag="xps")

                def pv(kcp, psb2, xps=xps, h=h):
                    for ip in range(2):
                        for nn in range(NQ):
                            nc.tensor.matmul(
                                xps[:, nn * 512 : (nn + 1) * 512],
                                lhsT=vsb[:, kcp, ip, h, :],
                                rhs=psb2[:, ip, nn * 512 : (nn + 1) * 512],
                                start=(kcp == 0 and ip == 0),
                                stop=(kcp == KCP - 1 and ip == 1),
                            )

                pvq = None  # pending PV, emitted one pair late to keep the
                # in-order PE from stalling on the exp+mask latency
                for kcp in range(KCP):
                    psb2 = W.tile([128, 2, sq], bf, tag="psb")
                    for ip in range(2):
                        kc = kcp * 2 + ip
                        sps = PJ.tile([128, sq], f32, tag="pj")
                        for nn in range(NQ):
                            nc.tensor.matmul(
                                sps[:, nn * 512 : (nn + 1) * 512],
                                lhsT=kTh[
                                    hoff : hoff + 64, h // 2, kc * 128 : (kc + 1) * 128
                                ],
                                rhs=qTh[
                                    hoff : hoff + 64, h // 2, nn * 512 : (nn + 1) * 512
                                ],
                                start=True,
                                stop=True,
                            )
                        nc.scalar.activation(
                            psb2[:, ip],
                            sps,
                            Exp,
                            scale=rdts[kc // KCL][:, kc % KCL : kc % KCL + 1],
                        )
                    # mask multiply over the pair, routed by (h, kcp)
                    mk = maskT[:, kcp * 2 : kcp * 2 + 2, :]
                    if kcp in POOL_KCPS:
                        nc.gpsimd.tensor_tensor(out=psb2, in0=psb2, in1=mk, op=mult)
                    else:
                        nc.vector.tensor_tensor(out=psb2, in0=psb2, in1=mk, op=mult)
                    if pvq is not None:
                        pv(*pvq)
                    pvq = (kcp, psb2)
                    # previous head's normalize multiply: its reciprocal and
                    # gpsimd broadcast were issued at head end and have had a
                    # full pair-step to complete, so this DVE op never stalls
                    if kcp == 0 and pend is not None:
                        ph, pxps, przb = pend
                        nc.vector.tensor_tensor(
                            out=xT[(ph % 2) * 64 : (ph % 2) * 64 + 64, ph // 2, :],
                            in0=pxps[0:dk, :],
                            in1=przb,
                            op=mult,
                        )
                        pend = None
                    # stream emission AFTER the attention ops so the in-order
                    # PE runs scores/PV ahead of stream matmuls awaiting DMA
                    if h == 0:
                        if kcp == 0:
                            q_delta(2)
                            k_block(1)
                            k_dma(2)
                        elif kcp == 1:
                            q_delta(3)
                            v_block(1)
                            v_dma(2)
                        elif kcp == 2:
                            k_block(2)
                            k_dma(3)
                            mask_g(2)
                        elif kcp == 3:
                            v_block(2)
                            v_dma(3)
                        elif kcp == 4:
                            k_block(3)
                            mask_g(3)
                        elif kcp == 5:
                            v_block(3)
                            nc.sync.dma_start(
                                wo_sb,
                                t["wo"].rearrange("(c p) m -> p c m", p=128),
                            )
                # flush the last pending PV, then issue this head's 1/Z
                # reciprocal + gpsimd broadcast; they overlap the next head's
                # first scores/exp, and the multiply is deferred to the next
                # head's first pair
                pv(*pvq)
                rz = W2.tile([1, sq], f32, tag="rz", bufs=2)
                nc.vector.reciprocal(rz, xps[dk : dk + 1, :])
                rzb = W2.tile([64, sq], f32, tag="rzb", bufs=2)
                nc.gpsimd.partition_broadcast(rzb, rz)
                pend = (h, xps, rzb)

            # final head's normalize
            ph, pxps, przb = pend
            nc.vector.tensor_tensor(
                out=xT[(ph % 2) * 64 : (ph % 2) * 64 + 64, ph // 2, :],
                in0=pxps[0:dk, :],
                in1=przb,
                op=mult,
            )

            # --- output projection (partial, this core's head dims) ---
            for qc in range(QC):
                yps = PJ.tile([128, d], f32, tag="pj")
                for c in range(C3):
                    for col in range(0, d, 512):
                        ncol = min(512, d - col)
                        nc.tensor.matmul(
                            yps[:, col : col + ncol],
                            lhsT=xT[:, c, qc * 128 : (qc + 1) * 128],
                            rhs=wo_sb[:, c, col : col + ncol],
                            start=(c == 0),
                            stop=(c == C3 - 1),
                        )
                ysb = W2.tile([128, d], bf, tag="ysb", bufs=2)
                nc.vector.tensor_copy(ysb, yps)
                nc.sync.dma_start(t["yp"][qc * 128 : (qc + 1) * 128, :], ysb)

    nc.compile()
    return nc


def _hilo(x):
    hi = x.astype(F8)
    lo = (x - hi.astype(np.float32)).astype(F8)
    return hi, lo


def _in_maps(query, key, value, mask, Wq, bq, Wk, bk, Wv, bv, Wo, Wd, bd, sq=SQ, dh=DH):
    query = np.asarray(query, np.float32)
    key = np.asarray(key, np.float32)
    value = np.asarray(value, np.float32)
    mask = np.asarray(mask)
    C6 = D // 128

    def stream_hilo(x):  # [S, D] -> [128, C6, 2, S] fp8 (hi, lo)
        xT = np.ascontiguousarray(x.T)              # [D, S]
        hi, lo = _hilo(xT)
        out = np.empty((128, C6, 2, S), F8)
        r = hi.reshape(C6, 128, S)
        out[:, :, 0] = r.transpose(1, 0, 2)
        out[:, :, 1] = lo.reshape(C6, 128, S).transpose(1, 0, 2)
        return out

    def w_hilo(w):  # [D, dh] -> [128, C6, 2, dh]
        hi, lo = _hilo(np.ascontiguousarray(w, np.float32))
        out = np.empty((128, C6, 2, w.shape[1]), F8)
        out[:, :, 0] = hi.reshape(C6, 128, -1).transpose(1, 0, 2)
        out[:, :, 1] = lo.reshape(C6, 128, -1).transpose(1, 0, 2)
        return out

    from kernel import WSCALE

    qhl = [stream_hilo(query[b]) for b in range(B)]
    khl = [stream_hilo(key[b]) for b in range(B)]
    v8 = [stream_hilo(value[b]) for b in range(B)]
    # weights pre-scaled by WSCALE before the fp8 hi/lo split so the hi part
    # stays out of e4m3's subnormal range; compensated in the bias ops
    wdf = np.ascontiguousarray(Wd, np.float32) * WSCALE  # [D, 1]
    wdh, wdl = _hilo(wdf)
    wd8 = np.zeros((128, C6, 4), F8)
    wd8[:, :, 0] = wdh.reshape(C6, 128).T
    wd8[:, :, 1] = wdh.reshape(C6, 128).T
    wd8[:, :, 2] = wdl.reshape(C6, 128).T
    wd8[:, :, 3] = wdl.reshape(C6, 128).T
    # DR2 rhs (wdl, wdl) makes delta the full 4-term product at no extra cost
    wqf = np.ascontiguousarray(Wq, np.float32) * WSCALE
    wkf = np.ascontiguousarray(Wk, np.float32) * WSCALE
    wvf = np.ascontiguousarray(Wv, np.float32) * WSCALE
    wob = np.ascontiguousarray(Wo).astype(BF16)
    bqf = np.ascontiguousarray(bq, np.float32) * WSCALE
    bkf = np.ascontiguousarray(bk, np.float32) * WSCALE
    bvf = np.ascontiguousarray(bv, np.float32)
    bdf = (np.ascontiguousarray(bd, np.float32) + 1.0) * WSCALE

    maps = []
    for c in range(NCORES):
        b, qh, hf = c // 4, (c // 2) % 2, c % 2
        qs = slice(qh * sq, (qh + 1) * sq)
        hs = slice(hf * dh, (hf + 1) * dh)
        # own q-half first in the stream so blocks [0, NBQ) are projected.
        # The same permutation is applied to the key axis everywhere
        # (khl, v8, maskT rows): attention sums over keys, so order is free
        # as long as delta, keys, values and mask rows agree.
        os_ = slice((1 - qh) * sq, (2 - qh) * sq)
        qcore = np.concatenate([qhl[b][:, :, :, qs], qhl[b][:, :, :, os_]], axis=3)
        kcore = np.concatenate([khl[b][:, :, :, qs], khl[b][:, :, :, os_]], axis=3)
        vcore = np.concatenate([v8[b][:, :, :, qs], v8[b][:, :, :, os_]], axis=3)
        mT = np.ascontiguousarray(mask[b, qs].T)  # [S(key), sq]
        mcore = np.concatenate([mT[qs], mT[os_]], axis=0)
        maps.append(
            {
                "qhl": np.ascontiguousarray(qcore),
                "khl": np.ascontiguousarray(kcore),
                "v8": np.ascontiguousarray(vcore),
                "maskT": np.ascontiguousarray(mcore).astype(BF16),
                "wq8": w_hilo(wqf[:, hs]),
                "wk8": w_hilo(wkf[:, hs]),
                "wv8": w_hilo(wvf[:, hs]),
                "wo": np.ascontiguousarray(wob[hs, :]),
                "wd8": wd8,
                "bq": np.ascontiguousarray(bqf[hs]),
                "bk": np.ascontiguousarray(bkf[hs]),
                "bv": np.ascontiguousarray(bvf[hs]),
                "bd": bdf,
            }
        )
    return maps


def kernel(query, key, value, mask, Wq, bq, Wk, bk, Wv, bv, Wo, bo, Wd, bd):
    from concourse.bass_utils import run_bass_kernel_spmd

    if "nc" not in _cache:
        _cache["nc"] = _build()
    nc = _cache["nc"]

    maps = _in_maps(query, key, value, mask, Wq, bq, Wk, bk, Wv, bv, Wo, Wd, bd)
    res = run_bass_kernel_spmd(nc, maps, core_ids=list(range(NCORES)))

    bof = np.asarray(bo, np.float32)
    y = np.empty((B, S, D), np.float32)
    for b in range(B):
        for qh in range(2):
            c0 = b * 4 + qh * 2
            y[b, qh * SQ : (qh + 1) * SQ] = (
                res.results[c0]["yp"].astype(np.float32)
                + res.results[c0 + 1]["yp"].astype(np.float32)
                + bof[None, :]
            )
    return y


# revision 64
# speedup vs baseline: 1.0866x; 1.0866x over previous
"""Trainium2 Bass kernel for MultiHeadedAttention with learned per-key-position scaling.

Sharding over 8 NeuronCores: batch(2) x q-half(2) x head-half(2), fully
independent SPMD (no collectives): each core streams the FULL query in fp8
hi/lo form and computes the per-key-position divisor delta locally.

Key techniques vs the v1 baseline:
  - Q/K projections run as fp8e4 DoubleRow matmuls on a hi+lo split of the
    inputs (3-term product W*x ~ Wh*xh + Wl*xh + Wh*xl): 0.75x PE cost and
    half the input DMA of f32; the PSUM accumulation stays f32-exact enough
    (~2e-4) that scores still use the f32r path.
  - V projection runs single-term fp8 DoubleRow (0.25x PE cost).
  - Attention probabilities stay bf16 (the softmax has no max-subtraction,
    so exp() values exceed fp8 range); a subset of mask multiplies is
    routed to GPSIMD/Pool to offload the DVE.
  - The softmax denominator broadcast uses gpsimd partition_broadcast,
    issued at head end with the normalize multiply deferred into the next
    head's first pair so its latency fully hides.

Host combines per-core partial outputs (sum over head-halves + bo).
"""

import sys

for _p in ("/opt/trn_rl_repo",):
    if _p not in sys.path:
        sys.path.insert(0, _p)

import numpy as np
import ml_dtypes

BF16 = ml_dtypes.bfloat16
F8 = ml_dtypes.float8_e4m3

B, S, D, H, DK = 2, 2048, 768, 12, 64
NCORES = 8
SQ = S // 2          # query rows per core
HH = H // 2          # heads per core
DH = HH * DK         # 384 head dims per core

# tunables
BW = 512             # stream block width (projection inputs)
POOL_KCPS = ()           # kc-pairs whose mask-mult runs on gpsimd (per head);
                         # empty: the ~4us gpsimd op latency stalls the
                         # in-order PE queue more than it saves on DVE
WSCALE = 32.0        # fp8 weight pre-scale (avoids e4m3 subnormal underflow)

_cache = {}


def _build(s=S, sq=SQ, hh=HH, d=D, dk=DK, n_qh=2):
    import concourse.bass as bass
    import concourse.mybir as mybir
    import concourse.tile as tile
    from concourse import bacc

    f32 = mybir.dt.float32
    f32r = mybir.dt.float32r
    bf = mybir.dt.bfloat16
    f8 = mybir.dt.float8e4
    DR = mybir.MatmulPerfMode.DoubleRow
    Exp = mybir.ActivationFunctionType.Exp
    mult = mybir.AluOpType.mult
    add = mybir.AluOpType.add
    amin = mybir.AluOpType.min
    amax = mybir.AluOpType.max

    dh = hh * dk
    KC = s // 128        # key-position chunks (16)
    KCP = KC // 2        # kc pairs (8)
    C6 = d // 128        # d_model chunks (6)
    C3 = dh // 128       # output-dim chunks per core (3)
    NQ = sq // 512       # 512-wide q column blocks (attention) (2)
    QC = sq // 128       # q row chunks for output projection (8)
    NBS = s // BW        # full-seq stream blocks (4)
    KCL = BW // 128      # kc chunks per stream block (4)

    nc = bacc.Bacc("TRN2", target_bir_lowering=False, debug=False, num_devices=NCORES)

    t = {}
    # hi/lo fp8 streams, full sequence; layouts match SBUF tiles exactly
    t["qhl"] = nc.dram_tensor("qhl", [128, C6, 2, s], f8, kind="ExternalInput").ap()
    t["khl"] = nc.dram_tensor("khl", [128, C6, 2, s], f8, kind="ExternalInput").ap()
    t["v8"] = nc.dram_tensor("v8", [128, C6, 2, s], f8, kind="ExternalInput").ap()
    t["maskT"] = nc.dram_tensor("maskT", [s, sq], bf, kind="ExternalInput").ap()
    t["wq8"] = nc.dram_tensor("wq8", [128, C6, 2, dh], f8, kind="ExternalInput").ap()
    t["wk8"] = nc.dram_tensor("wk8", [128, C6, 2, dh], f8, kind="ExternalInput").ap()
    t["wv8"] = nc.dram_tensor("wv8", [128, C6, 2, dh], f8, kind="ExternalInput").ap()
    t["wo"] = nc.dram_tensor("wo", [dh, d], bf, kind="ExternalInput").ap()
    # wd8 columns: [wdh, wdh, wdl, 0] so both delta DR matmuls stride cleanly
    t["wd8"] = nc.dram_tensor("wd8", [128, C6, 4], f8, kind="ExternalInput").ap()
    t["bq"] = nc.dram_tensor("bq", [dh], f32, kind="ExternalInput").ap()
    t["bk"] = nc.dram_tensor("bk", [dh], f32, kind="ExternalInput").ap()
    t["bv"] = nc.dram_tensor("bv", [dh], f32, kind="ExternalInput").ap()
    t["bd"] = nc.dram_tensor("bd", [1], f32, kind="ExternalInput").ap()
    t["yp"] = nc.dram_tensor("yp", [sq, d], bf, kind="ExternalOutput").ap()

    def bcast(ap, n):
        # broadcast a 1-D DRAM vector across n partitions
        return bass.AP(tensor=ap.tensor, offset=ap.offset, ap=[[0, n]] + list(ap.ap))

    def rep0(ap):
        # stride-0 doubling of a singleton middle dim: [p, 1, n] -> [p, 2(0), n]
        naps = [list(dd) for dd in ap.ap]
        assert naps[1][1] == 1
        naps[1] = [0, 2]
        return bass.AP(tensor=ap.tensor, offset=ap.offset, ap=naps)

    with tile.TileContext(nc) as tc:
        with (
            tc.tile_pool(name="persist", bufs=1) as P,
            tc.tile_pool(name="pj", bufs=3, space="PSUM") as PJ,
            tc.tile_pool(name="xpp", bufs=1, space="PSUM") as XPP,
            tc.tile_pool(name="work", bufs=3) as W,
            tc.tile_pool(name="work2", bufs=4) as W2,
            tc.tile_pool(name="load", bufs=4) as L,
            tc.tile_pool(name="loadv", bufs=2) as LV,
        ):
            maskT = P.tile([128, KC, sq], bf)
            vsb = P.tile([128, KCP, 2, hh, dk + 1], bf)
            qTh = P.tile([128, C3, sq], f32r)    # head pairs packed on partitions
            kTh = P.tile([128, C3, s], f32r)
            xTs = [P.tile([128, sq], bf, name=f"xT{i}") for i in range(C3)]
            wq_sb = P.tile([128, C6, 2, dh], f8)
            wk_sb = P.tile([128, C6, 2, dh], f8)
            wv_sb = P.tile([128, C6, 2, dh], f8)
            wo_sb = P.tile([128, C3, d], bf)
            wd_sb = P.tile([128, C6, 4], f8)
            bqc = P.tile([128, C3], f32)
            bkc = P.tile([128, C3], f32)
            bvb = P.tile([128, hh, dk], f32)
            bdb = P.tile([128, 1], f32)
            # per-stream-block reciprocal-delta tiles: written as each q block
            # lands so attention's exp never waits on the full q stream
            rdts = [P.tile([128, KCL], f32, name=f"rdt{i}") for i in range(NBS)]

            # warm the ACT exp table while DMAs stream
            dummy = W2.tile([1, 2], f32, tag="dummy", bufs=1)
            nc.vector.memset(dummy, 0.0)
            nc.scalar.activation(dummy, dummy, Exp, scale=1.0)

            nc.sync.dma_start(wd_sb, t["wd8"])
            nc.gpsimd.dma_start(bdb, bcast(t["bd"], 128))
            nc.sync.dma_start(bqc, t["bq"].rearrange("(c p) -> p c", p=128))
            nc.vector.memset(vsb[:, :, :, :, dk : dk + 1], 1.0)

            # q-half ordering is handled host-side: the own half occupies
            # stream blocks [0, sq//BW); the other half [sq//BW, NBS).  The
            # key axis (khl/v8/maskT rows) is permuted identically host-side
            # so delta columns line up with key chunks.
            NBQ = sq // BW  # own-half q blocks (2)

            # --- stream DMA helpers (issued in priority order below) ---
            ktiles, vtiles = {}, {}

            def k_dma(blk):
                kb = L.tile([128, C6, 2, BW], f8, tag="ldk", bufs=3)
                nc.sync.dma_start(kb, t["khl"][:, :, :, blk * BW : (blk + 1) * BW])
                ktiles[blk] = kb

            def v_dma(blk):
                vb = LV.tile([128, C6, 2, BW], f8, tag="ldv")
                nc.sync.dma_start(vb, t["v8"][:, :, :, blk * BW : (blk + 1) * BW])
                vtiles[blk] = vb

            def mask_g(g):
                nc.sync.dma_start(
                    maskT[:, g * (KC // 4) : (g + 1) * (KC // 4), :],
                    t["maskT"].rearrange("(kc p) q -> p kc q", p=128)[
                        :, g * (KC // 4) : (g + 1) * (KC // 4), :
                    ],
                )

            # --- Q stream: projection (own half) + delta partials (full S).
            # DMA priority: everything attention pairs 0-3 need lands first;
            # other-half q blocks (delta only) stream behind k1/v1.
            qtiles = []
            for blk in range(NBS):
                qb = L.tile([128, C6, 2, BW], f8, tag="ldq")
                nc.sync.dma_start(qb, t["qhl"][:, :, :, blk * BW : (blk + 1) * BW])
                qtiles.append(qb)
                if blk == 0:
                    nc.sync.dma_start(wq_sb, t["wq8"])
                    nc.sync.dma_start(wk_sb, t["wk8"])
                    nc.sync.dma_start(bkc, t["bk"].rearrange("(c p) -> p c", p=128))
                elif blk == 1:
                    k_dma(0)
                    nc.sync.dma_start(wv_sb, t["wv8"])
                    nc.gpsimd.dma_start(
                        bvb, bcast(t["bv"].rearrange("(h e) -> h e", h=hh), 128)
                    )
                    mask_g(0)
                    v_dma(0)
                    k_dma(1)
                    mask_g(1)
                    v_dma(1)

            def q_delta(blk):
                # delta partials: z = (qh+ql)@(wdh+wdl)  (per kc column)
                qb = qtiles[blk]
                dps = PJ.tile([128, KCL], f32, tag="pj", name=f"dps{blk}")
                for kcl in range(KCL):
                    for c in range(C6):
                        lhs_pair = qb[:, c, :, kcl * 128 : (kcl + 1) * 128]
                        nc.tensor.matmul(
                            dps[:, kcl : kcl + 1],
                            lhsT=lhs_pair,
                            rhs=wd_sb[:, c, 0:2].rearrange("p (i o) -> p i o", o=1),
                            start=(c == 0),
                            stop=False,
                            perf_mode=DR,
                        )
                        nc.tensor.matmul(
                            dps[:, kcl : kcl + 1],
                            lhsT=lhs_pair,
                            rhs=wd_sb[:, c, 2:4].rearrange("p (i o) -> p i o", o=1),
                            start=False,
                            stop=(c == C6 - 1),
                            perf_mode=DR,
                        )
                # this block's reciprocal delta (host bdb = WSCALE*(bd+1))
                dloc = W2.tile([128, KCL], f32, tag="dloc", bufs=1, name=f"dl{blk}")
                nc.vector.tensor_scalar(
                    out=dloc, in0=dps, scalar1=bdb, scalar2=1.0 / WSCALE,
                    op0=add, op1=mult,
                )
                nc.vector.tensor_scalar(
                    out=dloc, in0=dloc, scalar1=1.0, scalar2=9.0, op0=amax, op1=amin
                )
                nc.vector.reciprocal(rdts[blk], dloc)

            for blk in range(NBQ):
                qb = qtiles[blk]
                q_delta(blk)
                if True:
                    for m in range(C3):
                        qp = PJ.tile([128, BW], f32, tag="pj")
                        first = True
                        for c in range(C6):
                            nc.tensor.matmul(
                                qp,
                                lhsT=wq_sb[:, c, :, m * 128 : (m + 1) * 128],
                                rhs=rep0(qb[:, c, 0:1, :]),
                                start=first,
                                stop=False,
                                perf_mode=DR,
                            )
                            first = False
                        for c in range(0, C6, 2):
                            nc.tensor.matmul(
                                qp,
                                lhsT=wq_sb[:, c : c + 2, 0, m * 128 : (m + 1) * 128],
                                rhs=qb[:, c : c + 2, 1, :],
                                start=False,
                                stop=(c == C6 - 2),
                                perf_mode=DR,
                            )
                        nc.vector.tensor_scalar(
                            out=qTh[:, m, blk * BW : (blk + 1) * BW],
                            in0=qp,
                            scalar1=bqc[:, m : m + 1],
                            scalar2=1.0 / WSCALE,
                            op0=add,
                            op1=mult,
                        )

            # --- stream emitters, interleaved with head-0 attention below ---
            def k_block(blk):
                kb = ktiles.pop(blk)
                for m in range(C3):
                    kp = PJ.tile([128, BW], f32, tag="pj")
                    first = True
                    for c in range(C6):
                        nc.tensor.matmul(
                            kp,
                            lhsT=wk_sb[:, c, :, m * 128 : (m + 1) * 128],
                            rhs=rep0(kb[:, c, 0:1, :]),
                            start=first,
                            stop=False,
                            perf_mode=DR,
                        )
                        first = False
                    for c in range(0, C6, 2):
                        nc.tensor.matmul(
                            kp,
                            lhsT=wk_sb[:, c : c + 2, 0, m * 128 : (m + 1) * 128],
                            rhs=kb[:, c : c + 2, 1, :],
                            start=False,
                            stop=(c == C6 - 2),
                            perf_mode=DR,
                        )
                    nc.vector.tensor_scalar(
                        out=kTh[:, m, blk * BW : (blk + 1) * BW],
                        in0=kp,
                        scalar1=bkc[:, m : m + 1],
                        scalar2=1.0 / WSCALE,
                        op0=add,
                        op1=mult,
                    )

            def v_block(blk):
                vb = vtiles.pop(blk)
                for kcl in range(KCL):
                    kc = blk * KCL + kcl
                    kcp, ip = kc // 2, kc % 2
                    vp = PJ.tile([128, dh], f32, tag="pj")
                    # 3-term: (vh,vl)@(wvh,wvh) per chunk + (vh_c,vh_c+1)@(wvl_c,wvl_c+1)
                    for c in range(C6):
                        nc.tensor.matmul(
                            vp,
                            lhsT=vb[:, c, :, kcl * 128 : (kcl + 1) * 128],
                            rhs=rep0(wv_sb[:, c, 0:1, :]),
                            start=(c == 0),
                            stop=False,
                            perf_mode=DR,
                        )
                    for c in range(0, C6, 2):
                        nc.tensor.matmul(
                            vp,
                            lhsT=vb[:, c : c + 2, 0, kcl * 128 : (kcl + 1) * 128],
                            rhs=wv_sb[:, c : c + 2, 1, :],
                            start=False,
                            stop=(c == C6 - 2),
                            perf_mode=DR,
                        )
                    nc.vector.scalar_tensor_tensor(
                        out=vsb[:, kcp, ip, :, 0:dk],
                        in0=vp.rearrange("p (h e) -> p h e", h=hh),
                        scalar=1.0 / WSCALE,
                        in1=bvb,
                        op0=mult,
                        op1=add,
                    )

            # prologue: blocks 0/1 DMAs were issued with the q stream above;
            # v_block(0) is deferred into head 0 so its wv-weight wait never
            # blocks the PE queue ahead of the first scores
            k_block(0)

            # --- attention: 6 heads x 8 kc-pairs ---
            pend = None
            for h in range(hh):
                hoff = (h % 2) * 64
                xps = XPP.tile([dk + 1, sq], f32, tag="xps")

                def pv(kcp, psb2, xps=xps, h=h):
                    for ip in range(2):
                        for nn in range(NQ):
                            nc.tensor.matmul(
                                xps[:, nn * 512 : (nn + 1) * 512],
                                lhsT=vsb[:, kcp, ip, h, :],
                                rhs=psb2[:, ip, nn * 512 : (nn + 1) * 512],
                                start=(kcp == 0 and ip == 0),
                                stop=(kcp == KCP - 1 and ip == 1),
                            )

                pvq = []  # pending PVs, emitted two pairs late so the mask
                # has long completed and the in-order PE never stalls
                for kcp in range(KCP):
                    psb2 = W.tile([128, 2, sq], bf, tag="psb", bufs=4)
                    for ip in range(2):
                        kc = kcp * 2 + ip
                        sps = PJ.tile([128, sq], f32, tag="pj")
                        for nn in range(NQ):
                            nc.tensor.matmul(
                                sps[:, nn * 512 : (nn + 1) * 512],
                                lhsT=kTh[
                                    hoff : hoff + 64, h // 2, kc * 128 : (kc + 1) * 128
                                ],
                                rhs=qTh[
                                    hoff : hoff + 64, h // 2, nn * 512 : (nn + 1) * 512
                                ],
                                start=True,
                                stop=True,
                            )
                        nc.scalar.activation(
                            psb2[:, ip],
                            sps,
                            Exp,
                            scale=rdts[kc // KCL][:, kc % KCL : kc % KCL + 1],
                        )
                    # mask multiply over the pair, routed by (h, kcp)
                    mk = maskT[:, kcp * 2 : kcp * 2 + 2, :]
                    if kcp in POOL_KCPS:
                        nc.gpsimd.tensor_tensor(out=psb2, in0=psb2, in1=mk, op=mult)
                    else:
                        nc.vector.tensor_tensor(out=psb2, in0=psb2, in1=mk, op=mult)
                    pvq.append((kcp, psb2))
                    if len(pvq) > 2:
                        pv(*pvq.pop(0))
                    # previous head's normalize multiply: its reciprocal and
                    # gpsimd broadcast were issued at head end and have had a
                    # full pair-step to complete, so this DVE op never stalls
                    if kcp == 0 and pend is not None:
                        ph, pxps, przb = pend
                        nc.vector.tensor_tensor(
                            out=xTs[ph // 2][(ph % 2) * 64 : (ph % 2) * 64 + 64, :],
                            in0=pxps[0:dk, :],
                            in1=przb,
                            op=mult,
                        )
                        pend = None
                    # stream emission AFTER the attention ops so the in-order
                    # PE runs scores/PV ahead of stream matmuls awaiting DMA
                    if h == 0:
                        if kcp == 0:
                            v_block(0)
                            q_delta(2)
                            k_block(1)
                            k_dma(2)
                        elif kcp == 1:
                            q_delta(3)
                            v_block(1)
                            v_dma(2)
                        elif kcp == 2:
                            k_block(2)
                            k_dma(3)
                            mask_g(2)
                        elif kcp == 3:
                            v_block(2)
                            v_dma(3)
                        elif kcp == 4:
                            k_block(3)
                            mask_g(3)
                        elif kcp == 5:
                            v_block(3)
                            nc.sync.dma_start(
                                wo_sb,
                                t["wo"].rearrange("(c p) m -> p c m", p=128),
                            )
                # flush pending PVs, then issue this head's 1/Z reciprocal +
                # gpsimd broadcast; they overlap the next head's first
                # scores/exp, and the multiply is deferred to the next head
                while pvq:
                    pv(*pvq.pop(0))
                rz = W2.tile([1, sq], f32, tag="rz", bufs=2)
                nc.vector.reciprocal(rz, xps[dk : dk + 1, :])
                rzb = W2.tile([64, sq], f32, tag="rzb", bufs=2)
                nc.gpsimd.partition_broadcast(rzb, rz)
                pend = (h, xps, rzb)

            # final head's normalize
            ph, pxps, przb = pend
            nc.vector.tensor_tensor(
                out=xTs[ph // 2][(ph % 2) * 64 : (ph % 2) * 64 + 64, :],
                in0=pxps[0:dk, :],
                in1=przb,
                op=mult,
            )

            # --- output projection (partial, this core's head dims) ---
            # c-chunks 0/1 (heads 0-3) accumulate while the final head's
            # normalize chain drains; the closing c=2 matmuls trail 2 deep
            def yproj_c(qc, yps, c, start, stop):
                for col in range(0, d, 512):
                    ncol = min(512, d - col)
                    nc.tensor.matmul(
                        yps[:, col : col + ncol],
                        lhsT=xTs[c][:, qc * 128 : (qc + 1) * 128],
                        rhs=wo_sb[:, c, col : col + ncol],
                        start=start,
                        stop=stop,
                    )

            def yproj_close(qc, yps):
                yproj_c(qc, yps, C3 - 1, False, True)
                ysb = W2.tile([128, d], bf, tag="ysb", bufs=2)
                nc.vector.tensor_copy(ysb, yps)
                nc.sync.dma_start(t["yp"][qc * 128 : (qc + 1) * 128, :], ysb)

            yopen = []
            for qc in range(QC):
                yps = PJ.tile([128, d], f32, tag="pj")
                yproj_c(qc, yps, 0, True, False)
                yproj_c(qc, yps, 1, False, False)
                yopen.append((qc, yps))
                if len(yopen) > 2:
                    yproj_close(*yopen.pop(0))
            while yopen:
                yproj_close(*yopen.pop(0))

    nc.compile()
    return nc


def _hilo(x):
    hi = x.astype(F8)
    lo = (x - hi.astype(np.float32)).astype(F8)
    return hi, lo


def _in_maps(query, key, value, mask, Wq, bq, Wk, bk, Wv, bv, Wo, Wd, bd, sq=SQ, dh=DH):
    query = np.asarray(query, np.float32)
    key = np.asarray(key, np.float32)
    value = np.asarray(value, np.float32)
    mask = np.asarray(mask)
    C6 = D // 128

    def stream_hilo(x):  # [S, D] -> [128, C6, 2, S] fp8 (hi, lo)
        xT = np.ascontiguousarray(x.T)              # [D, S]
        hi, lo = _hilo(xT)
        out = np.empty((128, C6, 2, S), F8)
        r = hi.reshape(C6, 128, S)
        out[:, :, 0] = r.transpose(1, 0, 2)
        out[:, :, 1] = lo.reshape(C6, 128, S).transpose(1, 0, 2)
        return out

    def w_hilo(w):  # [D, dh] -> [128, C6, 2, dh]
        hi, lo = _hilo(np.ascontiguousarray(w, np.float32))
        out = np.empty((128, C6, 2, w.shape[1]), F8)
        out[:, :, 0] = hi.reshape(C6, 128, -1).transpose(1, 0, 2)
        out[:, :, 1] = lo.reshape(C6, 128, -1).transpose(1, 0, 2)
        return out

    from kernel import WSCALE

    qhl = [stream_hilo(query[b]) for b in range(B)]
    khl = [stream_hilo(key[b]) for b in range(B)]
    v8 = [stream_hilo(value[b]) for b in range(B)]
    # weights pre-scaled by WSCALE before the fp8 hi/lo split so the hi part
    # stays out of e4m3's subnormal range; compensated in the bias ops
    wdf = np.ascontiguousarray(Wd, np.float32) * WSCALE  # [D, 1]
    wdh, wdl = _hilo(wdf)
    wd8 = np.zeros((128, C6, 4), F8)
    wd8[:, :, 0] = wdh.reshape(C6, 128).T
    wd8[:, :, 1] = wdh.reshape(C6, 128).T
    wd8[:, :, 2] = wdl.reshape(C6, 128).T
    wd8[:, :, 3] = wdl.reshape(C6, 128).T
    # DR2 rhs (wdl, wdl) makes delta the full 4-term product at no extra cost
    wqf = np.ascontiguousarray(Wq, np.float32) * WSCALE
    wkf = np.ascontiguousarray(Wk, np.float32) * WSCALE
    wvf = np.ascontiguousarray(Wv, np.float32) * WSCALE
    wob = np.ascontiguousarray(Wo).astype(BF16)
    bqf = np.ascontiguousarray(bq, np.float32) * WSCALE
    bkf = np.ascontiguousarray(bk, np.float32) * WSCALE
    bvf = np.ascontiguousarray(bv, np.float32)
    bdf = (np.ascontiguousarray(bd, np.float32) + 1.0) * WSCALE

    maps = []
    for c in range(NCORES):
        b, qh, hf = c // 4, (c // 2) % 2, c % 2
        qs = slice(qh * sq, (qh + 1) * sq)
        hs = slice(hf * dh, (hf + 1) * dh)
        # own q-half first in the stream so blocks [0, NBQ) are projected.
        # The same permutation is applied to the key axis everywhere
        # (khl, v8, maskT rows): attention sums over keys, so order is free
        # as long as delta, keys, values and mask rows agree.
        os_ = slice((1 - qh) * sq, (2 - qh) * sq)
        qcore = np.concatenate([qhl[b][:, :, :, qs], qhl[b][:, :, :, os_]], axis=3)
        kcore = np.concatenate([khl[b][:, :, :, qs], khl[b][:, :, :, os_]], axis=3)
        vcore = np.concatenate([v8[b][:, :, :, qs], v8[b][:, :, :, os_]], axis=3)
        mT = np.ascontiguousarray(mask[b, qs].T)  # [S(key), sq]
        mcore = np.concatenate([mT[qs], mT[os_]], axis=0)
        maps.append(
            {
                "qhl": np.ascontiguousarray(qcore),
                "khl": np.ascontiguousarray(kcore),
                "v8": np.ascontiguousarray(vcore),
                "maskT": np.ascontiguousarray(mcore).astype(BF16),
                "wq8": w_hilo(wqf[:, hs]),
                "wk8": w_hilo(wkf[:, hs]),
                "wv8": w_hilo(wvf[:, hs]),
                "wo": np.ascontiguousarray(wob[hs, :]),
                "wd8": wd8,
                "bq": np.ascontiguousarray(bqf[hs]),
                "bk": np.ascontiguousarray(bkf[hs]),
                "bv": np.ascontiguousarray(bvf[hs]),
                "bd": bdf,
            }
        )
    return maps


def kernel(query, key, value, mask, Wq, bq, Wk, bk, Wv, bv, Wo, bo, Wd, bd):
    from concourse.bass_utils import run_bass_kernel_spmd

    if "nc" not in _cache:
        _cache["nc"] = _build()
    nc = _cache["nc"]

    maps = _in_maps(query, key, value, mask, Wq, bq, Wk, bk, Wv, bv, Wo, Wd, bd)
    res = run_bass_kernel_spmd(nc, maps, core_ids=list(range(NCORES)))

    bof = np.asarray(bo, np.float32)
    y = np.empty((B, S, D), np.float32)
    for b in range(B):
        for qh in range(2):
            c0 = b * 4 + qh * 2
            y[b, qh * SQ : (qh + 1) * SQ] = (
                res.results[c0]["yp"].astype(np.float32)
                + res.results[c0 + 1]["yp"].astype(np.float32)
                + bof[None, :]
            )
    return y


# revision 66
# speedup vs baseline: 1.1072x; 1.0190x over previous
"""Trainium2 Bass kernel for MultiHeadedAttention with learned per-key-position scaling.

Sharding over 8 NeuronCores: batch(2) x q-half(2) x head-half(2), fully
independent SPMD (no collectives): each core streams the FULL query in fp8
hi/lo form and computes the per-key-position divisor delta locally.

Key techniques vs the v1 baseline:
  - Q/K projections run as fp8e4 DoubleRow matmuls on a hi+lo split of the
    inputs (3-term product W*x ~ Wh*xh + Wl*xh + Wh*xl): 0.75x PE cost and
    half the input DMA of f32; the PSUM accumulation stays f32-exact enough
    (~2e-4) that scores still use the f32r path.
  - V projection runs single-term fp8 DoubleRow (0.25x PE cost).
  - Attention probabilities stay bf16 (the softmax has no max-subtraction,
    so exp() values exceed fp8 range); a subset of mask multiplies is
    routed to GPSIMD/Pool to offload the DVE.
  - The softmax denominator broadcast uses gpsimd partition_broadcast,
    issued at head end with the normalize multiply deferred into the next
    head's first pair so its latency fully hides.

Host combines per-core partial outputs (sum over head-halves + bo).
"""

import sys

for _p in ("/opt/trn_rl_repo",):
    if _p not in sys.path:
        sys.path.insert(0, _p)

import numpy as np
import ml_dtypes

BF16 = ml_dtypes.bfloat16
F8 = ml_dtypes.float8_e4m3

B, S, D, H, DK = 2, 2048, 768, 12, 64
NCORES = 8
SQ = S // 2          # query rows per core
HH = H // 2          # heads per core
DH = HH * DK         # 384 head dims per core

# tunables
BW = 512             # stream block width (projection inputs)
POOL_KCPS = ()           # kc-pairs whose mask-mult runs on gpsimd (per head);
                         # empty: the ~4us gpsimd op latency stalls the
                         # in-order PE queue more than it saves on DVE
WSCALE = 32.0        # fp8 weight pre-scale (avoids e4m3 subnormal underflow)

_cache = {}


def _build(s=S, sq=SQ, hh=HH, d=D, dk=DK, n_qh=2):
    import concourse.bass as bass
    import concourse.mybir as mybir
    import concourse.tile as tile
    from concourse import bacc

    f32 = mybir.dt.float32
    f32r = mybir.dt.float32r
    bf = mybir.dt.bfloat16
    f8 = mybir.dt.float8e4
    DR = mybir.MatmulPerfMode.DoubleRow
    Exp = mybir.ActivationFunctionType.Exp
    mult = mybir.AluOpType.mult
    add = mybir.AluOpType.add
    amin = mybir.AluOpType.min
    amax = mybir.AluOpType.max

    dh = hh * dk
    KC = s // 128        # key-position chunks (16)
    KCP = KC // 2        # kc pairs (8)
    C6 = d // 128        # d_model chunks (6)
    C3 = dh // 128       # output-dim chunks per core (3)
    NQ = sq // 512       # 512-wide q column blocks (attention) (2)
    QC = sq // 128       # q row chunks for output projection (8)
    NBS = s // BW        # full-seq stream blocks (4)
    KCL = BW // 128      # kc chunks per stream block (4)

    nc = bacc.Bacc("TRN2", target_bir_lowering=False, debug=False, num_devices=NCORES)

    t = {}
    # hi/lo fp8 streams, full sequence; layouts match SBUF tiles exactly
    t["qhl"] = nc.dram_tensor("qhl", [128, C6, 2, s], f8, kind="ExternalInput").ap()
    t["khl"] = nc.dram_tensor("khl", [128, C6, 2, s], f8, kind="ExternalInput").ap()
    t["v8"] = nc.dram_tensor("v8", [128, C6, 2, s], f8, kind="ExternalInput").ap()
    t["maskT"] = nc.dram_tensor("maskT", [s, sq], bf, kind="ExternalInput").ap()
    t["wq8"] = nc.dram_tensor("wq8", [128, C6, 2, dh], f8, kind="ExternalInput").ap()
    t["wk8"] = nc.dram_tensor("wk8", [128, C6, 2, dh], f8, kind="ExternalInput").ap()
    t["wv8"] = nc.dram_tensor("wv8", [128, C6, 2, dh], f8, kind="ExternalInput").ap()
    t["wo"] = nc.dram_tensor("wo", [dh, d], bf, kind="ExternalInput").ap()
    # wd8 columns: [wdh, wdh, wdl, 0] so both delta DR matmuls stride cleanly
    t["wd8"] = nc.dram_tensor("wd8", [128, C6, 4], f8, kind="ExternalInput").ap()
    t["bq"] = nc.dram_tensor("bq", [dh], f32, kind="ExternalInput").ap()
    t["bk"] = nc.dram_tensor("bk", [dh], f32, kind="ExternalInput").ap()
    t["bv"] = nc.dram_tensor("bv", [dh], f32, kind="ExternalInput").ap()
    t["bd"] = nc.dram_tensor("bd", [1], f32, kind="ExternalInput").ap()
    t["yp"] = nc.dram_tensor("yp", [sq, d], bf, kind="ExternalOutput").ap()

    def bcast(ap, n):
        # broadcast a 1-D DRAM vector across n partitions
        return bass.AP(tensor=ap.tensor, offset=ap.offset, ap=[[0, n]] + list(ap.ap))

    def rep0(ap):
        # stride-0 doubling of a singleton middle dim: [p, 1, n] -> [p, 2(0), n]
        naps = [list(dd) for dd in ap.ap]
        assert naps[1][1] == 1
        naps[1] = [0, 2]
        return bass.AP(tensor=ap.tensor, offset=ap.offset, ap=naps)

    with tile.TileContext(nc) as tc:
        with (
            tc.tile_pool(name="persist", bufs=1) as P,
            tc.tile_pool(name="pj", bufs=3, space="PSUM") as PJ,
            tc.tile_pool(name="xpp", bufs=1, space="PSUM") as XPP,
            tc.tile_pool(name="work", bufs=3) as W,
            tc.tile_pool(name="work2", bufs=4) as W2,
            tc.tile_pool(name="load", bufs=4) as L,
            tc.tile_pool(name="loadv", bufs=2) as LV,
        ):
            maskT = P.tile([128, KC, sq], bf)
            vsb = P.tile([128, KCP, 2, hh, dk + 1], bf)
            qTh = P.tile([128, C3, sq], f32r)    # head pairs packed on partitions
            kTh = P.tile([128, C3, s], f32r)
            xTs = [P.tile([128, sq], bf, name=f"xT{i}") for i in range(C3)]
            wq_sb = P.tile([128, C6, 2, dh], f8)
            wk_sb = P.tile([128, C6, 2, dh], f8)
            wv_sb = P.tile([128, C6, 2, dh], f8)
            wo_sb = P.tile([128, C3, d], bf)
            wd_sb = P.tile([128, C6, 4], f8)
            bqc = P.tile([128, C3], f32)
            bkc = P.tile([128, C3], f32)
            bvb = P.tile([128, hh, dk], f32)
            bdb = P.tile([128, 1], f32)
            # per-stream-block reciprocal-delta tiles: written as each q block
            # lands so attention's exp never waits on the full q stream
            rdts = [P.tile([128, KCL], f32, name=f"rdt{i}") for i in range(NBS)]

            # warm the ACT exp table while DMAs stream
            dummy = W2.tile([1, 2], f32, tag="dummy", bufs=1)
            nc.vector.memset(dummy, 0.0)
            nc.scalar.activation(dummy, dummy, Exp, scale=1.0)

            nc.sync.dma_start(wd_sb, t["wd8"])
            nc.gpsimd.dma_start(bdb, bcast(t["bd"], 128))
            nc.sync.dma_start(bqc, t["bq"].rearrange("(c p) -> p c", p=128))
            nc.vector.memset(vsb[:, :, :, :, dk : dk + 1], 1.0)

            # q-half ordering is handled host-side: the own half occupies
            # stream blocks [0, sq//BW); the other half [sq//BW, NBS).  The
            # key axis (khl/v8/maskT rows) is permuted identically host-side
            # so delta columns line up with key chunks.
            NBQ = sq // BW  # own-half q blocks (2)

            # --- stream DMA helpers (issued in priority order below) ---
            ktiles, vtiles = {}, {}

            def k_dma(blk):
                kb = L.tile([128, C6, 2, BW], f8, tag="ldk", bufs=3)
                nc.sync.dma_start(kb, t["khl"][:, :, :, blk * BW : (blk + 1) * BW])
                ktiles[blk] = kb

            def v_dma(blk):
                vb = LV.tile([128, C6, 2, BW], f8, tag="ldv")
                nc.sync.dma_start(vb, t["v8"][:, :, :, blk * BW : (blk + 1) * BW])
                vtiles[blk] = vb

            def mask_g(g):
                nc.sync.dma_start(
                    maskT[:, g * (KC // 4) : (g + 1) * (KC // 4), :],
                    t["maskT"].rearrange("(kc p) q -> p kc q", p=128)[
                        :, g * (KC // 4) : (g + 1) * (KC // 4), :
                    ],
                )

            # --- Q stream: projection (own half) + delta partials (full S).
            # DMA priority: everything attention pairs 0-3 need lands first;
            # other-half q blocks (delta only) stream behind k1/v1.
            qtiles = []
            for blk in range(NBS):
                qb = L.tile([128, C6, 2, BW], f8, tag="ldq")
                nc.sync.dma_start(qb, t["qhl"][:, :, :, blk * BW : (blk + 1) * BW])
                qtiles.append(qb)
                if blk == 0:
                    nc.sync.dma_start(wq_sb, t["wq8"])
                    nc.sync.dma_start(wk_sb, t["wk8"])
                    nc.sync.dma_start(bkc, t["bk"].rearrange("(c p) -> p c", p=128))
                elif blk == 1:
                    k_dma(0)
                    nc.sync.dma_start(wv_sb, t["wv8"])
                    nc.gpsimd.dma_start(
                        bvb, bcast(t["bv"].rearrange("(h e) -> h e", h=hh), 128)
                    )
                    mask_g(0)
                    v_dma(0)
                    k_dma(1)
                    mask_g(1)
                    v_dma(1)

            def q_delta(blk):
                # delta partials: z = (qh+ql)@(wdh+wdl)  (per kc column)
                qb = qtiles[blk]
                dps = PJ.tile([128, KCL], f32, tag="pj", name=f"dps{blk}")
                for kcl in range(KCL):
                    for c in range(C6):
                        lhs_pair = qb[:, c, :, kcl * 128 : (kcl + 1) * 128]
                        nc.tensor.matmul(
                            dps[:, kcl : kcl + 1],
                            lhsT=lhs_pair,
                            rhs=wd_sb[:, c, 0:2].rearrange("p (i o) -> p i o", o=1),
                            start=(c == 0),
                            stop=False,
                            perf_mode=DR,
                        )
                        nc.tensor.matmul(
                            dps[:, kcl : kcl + 1],
                            lhsT=lhs_pair,
                            rhs=wd_sb[:, c, 2:4].rearrange("p (i o) -> p i o", o=1),
                            start=False,
                            stop=(c == C6 - 1),
                            perf_mode=DR,
                        )
                # this block's reciprocal delta (host bdb = WSCALE*(bd+1))
                dloc = W2.tile([128, KCL], f32, tag="dloc", bufs=1, name=f"dl{blk}")
                nc.vector.tensor_scalar(
                    out=dloc, in0=dps, scalar1=bdb, scalar2=1.0 / WSCALE,
                    op0=add, op1=mult,
                )
                nc.vector.tensor_scalar(
                    out=dloc, in0=dloc, scalar1=1.0, scalar2=9.0, op0=amax, op1=amin
                )
                nc.vector.reciprocal(rdts[blk], dloc)

            for blk in range(NBQ):
                qb = qtiles[blk]
                q_delta(blk)
                if True:
                    for m in range(C3):
                        qp = PJ.tile([128, BW], f32, tag="pj")
                        first = True
                        for c in range(C6):
                            nc.tensor.matmul(
                                qp,
                                lhsT=wq_sb[:, c, :, m * 128 : (m + 1) * 128],
                                rhs=rep0(qb[:, c, 0:1, :]),
                                start=first,
                                stop=False,
                                perf_mode=DR,
                            )
                            first = False
                        for c in range(0, C6, 2):
                            nc.tensor.matmul(
                                qp,
                                lhsT=wq_sb[:, c : c + 2, 0, m * 128 : (m + 1) * 128],
                                rhs=qb[:, c : c + 2, 1, :],
                                start=False,
                                stop=(c == C6 - 2),
                                perf_mode=DR,
                            )
                        nc.vector.tensor_scalar(
                            out=qTh[:, m, blk * BW : (blk + 1) * BW],
                            in0=qp,
                            scalar1=bqc[:, m : m + 1],
                            scalar2=1.0 / WSCALE,
                            op0=add,
                            op1=mult,
                        )

            # --- stream emitters, interleaved with head-0 attention below ---
            def k_block(blk):
                kb = ktiles.pop(blk)
                for m in range(C3):
                    kp = PJ.tile([128, BW], f32, tag="pj")
                    first = True
                    for c in range(C6):
                        nc.tensor.matmul(
                            kp,
                            lhsT=wk_sb[:, c, :, m * 128 : (m + 1) * 128],
                            rhs=rep0(kb[:, c, 0:1, :]),
                            start=first,
                            stop=False,
                            perf_mode=DR,
                        )
                        first = False
                    for c in range(0, C6, 2):
                        nc.tensor.matmul(
                            kp,
                            lhsT=wk_sb[:, c : c + 2, 0, m * 128 : (m + 1) * 128],
                            rhs=kb[:, c : c + 2, 1, :],
                            start=False,
                            stop=(c == C6 - 2),
                            perf_mode=DR,
                        )
                    nc.vector.tensor_scalar(
                        out=kTh[:, m, blk * BW : (blk + 1) * BW],
                        in0=kp,
                        scalar1=bkc[:, m : m + 1],
                        scalar2=1.0 / WSCALE,
                        op0=add,
                        op1=mult,
                    )

            def v_block(blk):
                vb = vtiles.pop(blk)
                for kcl in range(KCL):
                    kc = blk * KCL + kcl
                    kcp, ip = kc // 2, kc % 2
                    vp = PJ.tile([128, dh], f32, tag="pj")
                    # 3-term: (vh,vl)@(wvh,wvh) per chunk + (vh_c,vh_c+1)@(wvl_c,wvl_c+1)
                    for c in range(C6):
                        nc.tensor.matmul(
                            vp,
                            lhsT=vb[:, c, :, kcl * 128 : (kcl + 1) * 128],
                            rhs=rep0(wv_sb[:, c, 0:1, :]),
                            start=(c == 0),
                            stop=False,
                            perf_mode=DR,
                        )
                    for c in range(0, C6, 2):
                        nc.tensor.matmul(
                            vp,
                            lhsT=vb[:, c : c + 2, 0, kcl * 128 : (kcl + 1) * 128],
                            rhs=wv_sb[:, c : c + 2, 1, :],
                            start=False,
                            stop=(c == C6 - 2),
                            perf_mode=DR,
                        )
                    nc.vector.scalar_tensor_tensor(
                        out=vsb[:, kcp, ip, :, 0:dk],
                        in0=vp.rearrange("p (h e) -> p h e", h=hh),
                        scalar=1.0 / WSCALE,
                        in1=bvb,
                        op0=mult,
                        op1=add,
                    )

            # prologue: blocks 0/1 DMAs were issued with the q stream above;
            # v_block(0) is deferred into head 0 so its wv-weight wait never
            # blocks the PE queue ahead of the first scores
            k_block(0)

            # --- attention: 6 heads x 8 kc-pairs ---
            def pv(kcp, psb2, xps, h):
                for ip in range(2):
                    for nn in range(NQ):
                        nc.tensor.matmul(
                            xps[:, nn * 512 : (nn + 1) * 512],
                            lhsT=vsb[:, kcp, ip, h, :],
                            rhs=psb2[:, ip, nn * 512 : (nn + 1) * 512],
                            start=(kcp == 0 and ip == 0),
                            stop=(kcp == KCP - 1 and ip == 1),
                        )

            def pop_pv():
                # emit the oldest pending PV; when it closes a head's
                # accumulation, chain that head's 1/Z reciprocal + broadcast
                pkcp, ppsb2, pxps, ph2 = pvq.pop(0)
                pv(pkcp, ppsb2, pxps, ph2)
                if pkcp == KCP - 1:
                    rz = W2.tile([1, sq], f32, tag="rz", bufs=2)
                    nc.vector.reciprocal(rz, pxps[dk : dk + 1, :])
                    rzb = W2.tile([64, sq], f32, tag="rzb", bufs=2)
                    nc.gpsimd.partition_broadcast(rzb, rz)
                    pend.append((ph2, pxps, rzb))

            # pending PVs, emitted two pairs late (carrying across head
            # boundaries) so the mask latency and the normalize chain stay
            # off the in-order PE's critical path
            pvq = []
            pend = []
            for h in range(hh):
                hoff = (h % 2) * 64
                xps = XPP.tile([dk + 1, sq], f32, tag="xps")
                for kcp in range(KCP):
                    psb2 = W.tile([128, 2, sq], bf, tag="psb", bufs=4)
                    for ip in range(2):
                        kc = kcp * 2 + ip
                        sps = PJ.tile([128, sq], f32, tag="pj")
                        for nn in range(NQ):
                            nc.tensor.matmul(
                                sps[:, nn * 512 : (nn + 1) * 512],
                                lhsT=kTh[
                                    hoff : hoff + 64, h // 2, kc * 128 : (kc + 1) * 128
                                ],
                                rhs=qTh[
                                    hoff : hoff + 64, h // 2, nn * 512 : (nn + 1) * 512
                                ],
                                start=True,
                                stop=True,
                            )
                        nc.scalar.activation(
                            psb2[:, ip],
                            sps,
                            Exp,
                            scale=rdts[kc // KCL][:, kc % KCL : kc % KCL + 1],
                        )
                    # mask multiply over the pair, routed by (h, kcp)
                    mk = maskT[:, kcp * 2 : kcp * 2 + 2, :]
                    if kcp in POOL_KCPS:
                        nc.gpsimd.tensor_tensor(out=psb2, in0=psb2, in1=mk, op=mult)
                    else:
                        nc.vector.tensor_tensor(out=psb2, in0=psb2, in1=mk, op=mult)
                    # previous head's normalize multiply (DVE) must precede
                    # this head's first PV (same single xps slot)
                    if pend:
                        ph, pxps, przb = pend.pop(0)
                        nc.vector.tensor_tensor(
                            out=xTs[ph // 2][(ph % 2) * 64 : (ph % 2) * 64 + 64, :],
                            in0=pxps[0:dk, :],
                            in1=przb,
                            op=mult,
                        )
                    pvq.append((kcp, psb2, xps, h))
                    if len(pvq) > 2:
                        pop_pv()
                    # stream emission AFTER the attention ops so the in-order
                    # PE runs scores/PV ahead of stream matmuls awaiting DMA
                    if h == 0:
                        if kcp == 0:
                            v_block(0)
                            q_delta(2)
                            k_block(1)
                            k_dma(2)
                        elif kcp == 1:
                            q_delta(3)
                            v_block(1)
                            v_dma(2)
                        elif kcp == 2:
                            k_block(2)
                            k_dma(3)
                            mask_g(2)
                        elif kcp == 3:
                            v_block(2)
                            v_dma(3)
                        elif kcp == 4:
                            k_block(3)
                            mask_g(3)
                        elif kcp == 5:
                            v_block(3)
                            nc.sync.dma_start(
                                wo_sb,
                                t["wo"].rearrange("(c p) m -> p c m", p=128),
                            )
            # drain remaining PVs and normalizes
            while pvq:
                pop_pv()
            while pend:
                ph, pxps, przb = pend.pop(0)
                nc.vector.tensor_tensor(
                    out=xTs[ph // 2][(ph % 2) * 64 : (ph % 2) * 64 + 64, :],
                    in0=pxps[0:dk, :],
                    in1=przb,
                    op=mult,
                )

            # --- output projection (partial, this core's head dims) ---
            # c-chunks 0/1 (heads 0-3) accumulate while the final head's
            # normalize chain drains; the closing c=2 matmuls trail 2 deep
            def yproj_c(qc, yps, c, start, stop):
                for col in range(0, d, 512):
                    ncol = min(512, d - col)
                    nc.tensor.matmul(
                        yps[:, col : col + ncol],
                        lhsT=xTs[c][:, qc * 128 : (qc + 1) * 128],
                        rhs=wo_sb[:, c, col : col + ncol],
                        start=start,
                        stop=stop,
                    )

            def yproj_close(qc, yps):
                yproj_c(qc, yps, C3 - 1, False, True)
                ysb = W2.tile([128, d], bf, tag="ysb", bufs=4)
                # both ACT and DVE are idle in the tail: split the copies
                if qc % 2 == 0:
                    nc.scalar.copy(ysb, yps)
                else:
                    nc.vector.tensor_copy(ysb, yps)
                nc.sync.dma_start(t["yp"][qc * 128 : (qc + 1) * 128, :], ysb)

            yopen = []
            for qc in range(QC):
                yps = PJ.tile([128, d], f32, tag="pj")
                yproj_c(qc, yps, 0, True, False)
                yproj_c(qc, yps, 1, False, False)
                yopen.append((qc, yps))
                if len(yopen) > 2:
                    yproj_close(*yopen.pop(0))
            while yopen:
                yproj_close(*yopen.pop(0))

    nc.compile()
    return nc


def _hilo(x):
    hi = x.astype(F8)
    lo = (x - hi.astype(np.float32)).astype(F8)
    return hi, lo


def _in_maps(query, key, value, mask, Wq, bq, Wk, bk, Wv, bv, Wo, Wd, bd, sq=SQ, dh=DH):
    query = np.asarray(query, np.float32)
    key = np.asarray(key, np.float32)
    value = np.asarray(value, np.float32)
    mask = np.asarray(mask)
    C6 = D // 128

    def stream_hilo(x):  # [S, D] -> [128, C6, 2, S] fp8 (hi, lo)
        xT = np.ascontiguousarray(x.T)              # [D, S]
        hi, lo = _hilo(xT)
        out = np.empty((128, C6, 2, S), F8)
        r = hi.reshape(C6, 128, S)
        out[:, :, 0] = r.transpose(1, 0, 2)
        out[:, :, 1] = lo.reshape(C6, 128, S).transpose(1, 0, 2)
        return out

    def w_hilo(w):  # [D, dh] -> [128, C6, 2, dh]
        hi, lo = _hilo(np.ascontiguousarray(w, np.float32))
        out = np.empty((128, C6, 2, w.shape[1]), F8)
        out[:, :, 0] = hi.reshape(C6, 128, -1).transpose(1, 0, 2)
        out[:, :, 1] = lo.reshape(C6, 128, -1).transpose(1, 0, 2)
        return out

    from kernel import WSCALE

    qhl = [stream_hilo(query[b]) for b in range(B)]
    khl = [stream_hilo(key[b]) for b in range(B)]
    v8 = [stream_hilo(value[b]) for b in range(B)]
    # weights pre-scaled by WSCALE before the fp8 hi/lo split so the hi part
    # stays out of e4m3's subnormal range; compensated in the bias ops
    wdf = np.ascontiguousarray(Wd, np.float32) * WSCALE  # [D, 1]
    wdh, wdl = _hilo(wdf)
    wd8 = np.zeros((128, C6, 4), F8)
    wd8[:, :, 0] = wdh.reshape(C6, 128).T
    wd8[:, :, 1] = wdh.reshape(C6, 128).T
    wd8[:, :, 2] = wdl.reshape(C6, 128).T
    wd8[:, :, 3] = wdl.reshape(C6, 128).T
    # DR2 rhs (wdl, wdl) makes delta the full 4-term product at no extra cost
    wqf = np.ascontiguousarray(Wq, np.float32) * WSCALE
    wkf = np.ascontiguousarray(Wk, np.float32) * WSCALE
    wvf = np.ascontiguousarray(Wv, np.float32) * WSCALE
    wob = np.ascontiguousarray(Wo).astype(BF16)
    bqf = np.ascontiguousarray(bq, np.float32) * WSCALE
    bkf = np.ascontiguousarray(bk, np.float32) * WSCALE
    bvf = np.ascontiguousarray(bv, np.float32)
    bdf = (np.ascontiguousarray(bd, np.float32) + 1.0) * WSCALE

    maps = []
    for c in range(NCORES):
        b, qh, hf = c // 4, (c // 2) % 2, c % 2
        qs = slice(qh * sq, (qh + 1) * sq)
        hs = slice(hf * dh, (hf + 1) * dh)
        # own q-half first in the stream so blocks [0, NBQ) are projected.
        # The same permutation is applied to the key axis everywhere
        # (khl, v8, maskT rows): attention sums over keys, so order is free
        # as long as delta, keys, values and mask rows agree.
        os_ = slice((1 - qh) * sq, (2 - qh) * sq)
        qcore = np.concatenate([qhl[b][:, :, :, qs], qhl[b][:, :, :, os_]], axis=3)
        kcore = np.concatenate([khl[b][:, :, :, qs], khl[b][:, :, :, os_]], axis=3)
        vcore = np.concatenate([v8[b][:, :, :, qs], v8[b][:, :, :, os_]], axis=3)
        mT = np.ascontiguousarray(mask[b, qs].T)  # [S(key), sq]
        mcore = np.concatenate([mT[qs], mT[os_]], axis=0)
        maps.append(
            {
                "qhl": np.ascontiguousarray(qcore),
                "khl": np.ascontiguousarray(kcore),
                "v8": np.ascontiguousarray(vcore),
                "maskT": np.ascontiguousarray(mcore).astype(BF16),
                "wq8": w_hilo(wqf[:, hs]),
                "wk8": w_hilo(wkf[:, hs]),
                "wv8": w_hilo(wvf[:, hs]),
                "wo": np.ascontiguousarray(wob[hs, :]),
                "wd8": wd8,
                "bq": np.ascontiguousarray(bqf[hs]),
                "bk": np.ascontiguousarray(bkf[hs]),
                "bv": np.ascontiguousarray(bvf[hs]),
                "bd": bdf,
            }
        )
    return maps


def kernel(query, key, value, mask, Wq, bq, Wk, bk, Wv, bv, Wo, bo, Wd, bd):
    from concourse.bass_utils import run_bass_kernel_spmd

    if "nc" not in _cache:
        _cache["nc"] = _build()
    nc = _cache["nc"]

    maps = _in_maps(query, key, value, mask, Wq, bq, Wk, bk, Wv, bv, Wo, Wd, bd)
    res = run_bass_kernel_spmd(nc, maps, core_ids=list(range(NCORES)))

    bof = np.asarray(bo, np.float32)
    y = np.empty((B, S, D), np.float32)
    for b in range(B):
        for qh in range(2):
            c0 = b * 4 + qh * 2
            y[b, qh * SQ : (qh + 1) * SQ] = (
                res.results[c0]["yp"].astype(np.float32)
                + res.results[c0 + 1]["yp"].astype(np.float32)
                + bof[None, :]
            )
    return y


# revision 69
# speedup vs baseline: 1.1355x; 1.0255x over previous
"""Trainium2 Bass kernel for MultiHeadedAttention with learned per-key-position scaling.

Sharding over 8 NeuronCores: batch(2) x q-half(2) x head-half(2), fully
independent SPMD (no collectives): each core streams the FULL query in fp8
hi/lo form and computes the per-key-position divisor delta locally.

Key techniques vs the v1 baseline:
  - Q/K projections run as fp8e4 DoubleRow matmuls on a hi+lo split of the
    inputs (3-term product W*x ~ Wh*xh + Wl*xh + Wh*xl): 0.75x PE cost and
    half the input DMA of f32; the PSUM accumulation stays f32-exact enough
    (~2e-4) that scores still use the f32r path.
  - V projection runs single-term fp8 DoubleRow (0.25x PE cost).
  - Attention probabilities stay bf16 (the softmax has no max-subtraction,
    so exp() values exceed fp8 range); a subset of mask multiplies is
    routed to GPSIMD/Pool to offload the DVE.
  - The softmax denominator broadcast uses gpsimd partition_broadcast,
    issued at head end with the normalize multiply deferred into the next
    head's first pair so its latency fully hides.

Host combines per-core partial outputs (sum over head-halves + bo).
"""

import sys

for _p in ("/opt/trn_rl_repo",):
    if _p not in sys.path:
        sys.path.insert(0, _p)

import numpy as np
import ml_dtypes

BF16 = ml_dtypes.bfloat16
F8 = ml_dtypes.float8_e4m3

B, S, D, H, DK = 2, 2048, 768, 12, 64
NCORES = 8
SQ = S // 2          # query rows per core
HH = H // 2          # heads per core
DH = HH * DK         # 384 head dims per core

# tunables
BW = 512             # stream block width (projection inputs)
POOL_KCPS = ()           # kc-pairs whose mask-mult runs on gpsimd (per head);
                         # empty: the ~4us gpsimd op latency stalls the
                         # in-order PE queue more than it saves on DVE
WSCALE = 32.0        # fp8 weight pre-scale (avoids e4m3 subnormal underflow)

_cache = {}


def _build(s=S, sq=SQ, hh=HH, d=D, dk=DK, n_qh=2):
    import concourse.bass as bass
    import concourse.mybir as mybir
    import concourse.tile as tile
    from concourse import bacc

    f32 = mybir.dt.float32
    f32r = mybir.dt.float32r
    bf = mybir.dt.bfloat16
    f8 = mybir.dt.float8e4
    DR = mybir.MatmulPerfMode.DoubleRow
    Exp = mybir.ActivationFunctionType.Exp
    mult = mybir.AluOpType.mult
    add = mybir.AluOpType.add
    amin = mybir.AluOpType.min
    amax = mybir.AluOpType.max

    dh = hh * dk
    KC = s // 128        # key-position chunks (16)
    KCP = KC // 2        # kc pairs (8)
    C6 = d // 128        # d_model chunks (6)
    C3 = dh // 128       # output-dim chunks per core (3)
    NQ = sq // 512       # 512-wide q column blocks (attention) (2)
    QC = sq // 128       # q row chunks for output projection (8)
    NBS = s // BW        # full-seq stream blocks (4)
    KCL = BW // 128      # kc chunks per stream block (4)

    nc = bacc.Bacc("TRN2", target_bir_lowering=False, debug=False, num_devices=NCORES)

    t = {}
    # hi/lo fp8 streams, full sequence; layouts match SBUF tiles exactly
    t["qhl"] = nc.dram_tensor("qhl", [128, C6, 2, s], f8, kind="ExternalInput").ap()
    t["khl"] = nc.dram_tensor("khl", [128, C6, 2, s], f8, kind="ExternalInput").ap()
    t["v8"] = nc.dram_tensor("v8", [128, C6, 2, s], f8, kind="ExternalInput").ap()
    t["maskT"] = nc.dram_tensor("maskT", [s, sq], bf, kind="ExternalInput").ap()
    t["wq8"] = nc.dram_tensor("wq8", [128, C6, 2, dh], f8, kind="ExternalInput").ap()
    t["wk8"] = nc.dram_tensor("wk8", [128, C6, 2, dh], f8, kind="ExternalInput").ap()
    t["wv8"] = nc.dram_tensor("wv8", [128, C6, 2, dh], f8, kind="ExternalInput").ap()
    t["wo"] = nc.dram_tensor("wo", [dh, d], bf, kind="ExternalInput").ap()
    # wd8 columns: [wdh, wdh, wdl, 0] so both delta DR matmuls stride cleanly
    t["wd8"] = nc.dram_tensor("wd8", [128, C6, 4], f8, kind="ExternalInput").ap()
    t["bq"] = nc.dram_tensor("bq", [dh], f32, kind="ExternalInput").ap()
    t["bk"] = nc.dram_tensor("bk", [dh], f32, kind="ExternalInput").ap()
    t["bv"] = nc.dram_tensor("bv", [dh], f32, kind="ExternalInput").ap()
    t["bd"] = nc.dram_tensor("bd", [1], f32, kind="ExternalInput").ap()
    t["yp"] = nc.dram_tensor("yp", [sq, d], bf, kind="ExternalOutput").ap()

    def bcast(ap, n):
        # broadcast a 1-D DRAM vector across n partitions
        return bass.AP(tensor=ap.tensor, offset=ap.offset, ap=[[0, n]] + list(ap.ap))

    def rep0(ap):
        # stride-0 doubling of a singleton middle dim: [p, 1, n] -> [p, 2(0), n]
        naps = [list(dd) for dd in ap.ap]
        assert naps[1][1] == 1
        naps[1] = [0, 2]
        return bass.AP(tensor=ap.tensor, offset=ap.offset, ap=naps)

    with tile.TileContext(nc) as tc:
        with (
            tc.tile_pool(name="persist", bufs=1) as P,
            tc.tile_pool(name="pj", bufs=3, space="PSUM") as PJ,
            tc.tile_pool(name="xpp", bufs=1, space="PSUM") as XPP,
            tc.tile_pool(name="work", bufs=3) as W,
            tc.tile_pool(name="work2", bufs=4) as W2,
            tc.tile_pool(name="load", bufs=4) as L,
            tc.tile_pool(name="loadv", bufs=2) as LV,
        ):
            maskT = P.tile([128, KC, sq], bf)
            vsb = P.tile([128, KCP, 2, hh, dk + 1], bf)
            qTh = P.tile([128, C3, sq], f32r)    # head pairs packed on partitions
            kTh = P.tile([128, C3, s], f32r)
            xTs = [P.tile([128, sq], bf, name=f"xT{i}") for i in range(C3)]
            wq_sb = P.tile([128, C6, 2, dh], f8)
            wk_sb = P.tile([128, C6, 2, dh], f8)
            wv_sb = P.tile([128, C6, 2, dh], f8)
            wo_sb = P.tile([128, C3, d], bf)
            wd_sb = P.tile([128, C6, 4], f8)
            bqc = P.tile([128, C3], f32)
            bkc = P.tile([128, C3], f32)
            bvb = P.tile([128, hh, dk], f32)
            bdb = P.tile([128, 1], f32)
            # per-stream-block reciprocal-delta tiles: written as each q block
            # lands so attention's exp never waits on the full q stream
            rdts = [P.tile([128, KCL], f32, name=f"rdt{i}") for i in range(NBS)]

            # warm the ACT exp table while DMAs stream
            dummy = W2.tile([1, 2], f32, tag="dummy", bufs=1)
            nc.vector.memset(dummy, 0.0)
            nc.scalar.activation(dummy, dummy, Exp, scale=1.0)

            nc.sync.dma_start(wd_sb, t["wd8"])
            nc.gpsimd.dma_start(bdb, bcast(t["bd"], 128))
            nc.sync.dma_start(bqc, t["bq"].rearrange("(c p) -> p c", p=128))
            nc.vector.memset(vsb[:, :, :, :, dk : dk + 1], 1.0)

            # q-half ordering is handled host-side: the own half occupies
            # stream blocks [0, sq//BW); the other half [sq//BW, NBS).  The
            # key axis (khl/v8/maskT rows) is permuted identically host-side
            # so delta columns line up with key chunks.
            NBQ = sq // BW  # own-half q blocks (2)

            # --- stream DMA helpers (issued in priority order below) ---
            ktiles, vtiles = {}, {}

            def k_dma(blk):
                kb = L.tile([128, C6, 2, BW], f8, tag="ldk", bufs=3)
                nc.sync.dma_start(kb, t["khl"][:, :, :, blk * BW : (blk + 1) * BW])
                ktiles[blk] = kb

            def v_dma(blk):
                vb = LV.tile([128, C6, 2, BW], f8, tag="ldv")
                nc.sync.dma_start(vb, t["v8"][:, :, :, blk * BW : (blk + 1) * BW])
                vtiles[blk] = vb

            def mask_g(g):
                nc.sync.dma_start(
                    maskT[:, g * (KC // 4) : (g + 1) * (KC // 4), :],
                    t["maskT"].rearrange("(kc p) q -> p kc q", p=128)[
                        :, g * (KC // 4) : (g + 1) * (KC // 4), :
                    ],
                )

            # --- Q stream: projection (own half) + delta partials (full S).
            # DMA priority: everything attention pairs 0-3 need lands first;
            # other-half q blocks (delta only) stream behind k1/v1.
            qtiles = []
            for blk in range(NBS):
                qb = L.tile([128, C6, 2, BW], f8, tag="ldq")
                nc.sync.dma_start(qb, t["qhl"][:, :, :, blk * BW : (blk + 1) * BW])
                qtiles.append(qb)
                if blk == 0:
                    nc.sync.dma_start(wq_sb, t["wq8"])
                    nc.sync.dma_start(wk_sb, t["wk8"])
                    nc.sync.dma_start(bkc, t["bk"].rearrange("(c p) -> p c", p=128))
                elif blk == 1:
                    k_dma(0)
                    nc.sync.dma_start(wv_sb, t["wv8"])
                    nc.gpsimd.dma_start(
                        bvb, bcast(t["bv"].rearrange("(h e) -> h e", h=hh), 128)
                    )
                    mask_g(0)
                    v_dma(0)
                    k_dma(1)
                    mask_g(1)
                    v_dma(1)

            def q_delta(blk):
                # delta partials: z = (qh+ql)@(wdh+wdl)  (per kc column)
                qb = qtiles[blk]
                dps = PJ.tile([128, KCL], f32, tag="pj", name=f"dps{blk}")
                for kcl in range(KCL):
                    for c in range(C6):
                        lhs_pair = qb[:, c, :, kcl * 128 : (kcl + 1) * 128]
                        nc.tensor.matmul(
                            dps[:, kcl : kcl + 1],
                            lhsT=lhs_pair,
                            rhs=wd_sb[:, c, 0:2].rearrange("p (i o) -> p i o", o=1),
                            start=(c == 0),
                            stop=False,
                            perf_mode=DR,
                        )
                        nc.tensor.matmul(
                            dps[:, kcl : kcl + 1],
                            lhsT=lhs_pair,
                            rhs=wd_sb[:, c, 2:4].rearrange("p (i o) -> p i o", o=1),
                            start=False,
                            stop=(c == C6 - 1),
                            perf_mode=DR,
                        )
                # this block's reciprocal delta (host bdb = WSCALE*(bd+1))
                dloc = W2.tile([128, KCL], f32, tag="dloc", bufs=1, name=f"dl{blk}")
                nc.vector.tensor_scalar(
                    out=dloc, in0=dps, scalar1=bdb, scalar2=1.0 / WSCALE,
                    op0=add, op1=mult,
                )
                nc.vector.tensor_scalar(
                    out=dloc, in0=dloc, scalar1=1.0, scalar2=9.0, op0=amax, op1=amin
                )
                nc.vector.reciprocal(rdts[blk], dloc)

            for blk in range(NBQ):
                qb = qtiles[blk]
                q_delta(blk)
                if True:
                    for m in range(C3):
                        qp = PJ.tile([128, BW], f32, tag="pj")
                        first = True
                        for c in range(C6):
                            nc.tensor.matmul(
                                qp,
                                lhsT=wq_sb[:, c, :, m * 128 : (m + 1) * 128],
                                rhs=rep0(qb[:, c, 0:1, :]),
                                start=first,
                                stop=False,
                                perf_mode=DR,
                            )
                            first = False
                        for c in range(0, C6, 2):
                            nc.tensor.matmul(
                                qp,
                                lhsT=wq_sb[:, c : c + 2, 0, m * 128 : (m + 1) * 128],
                                rhs=qb[:, c : c + 2, 1, :],
                                start=False,
                                stop=(c == C6 - 2),
                                perf_mode=DR,
                            )
                        nc.vector.tensor_scalar(
                            out=qTh[:, m, blk * BW : (blk + 1) * BW],
                            in0=qp,
                            scalar1=bqc[:, m : m + 1],
                            scalar2=1.0 / WSCALE,
                            op0=add,
                            op1=mult,
                        )

            # --- stream emitters, interleaved with head-0 attention below ---
            def k_block_m(blk, m):
                kb = ktiles[blk]
                if True:
                    kp = PJ.tile([128, BW], f32, tag="pj")
                    first = True
                    for c in range(C6):
                        nc.tensor.matmul(
                            kp,
                            lhsT=wk_sb[:, c, :, m * 128 : (m + 1) * 128],
                            rhs=rep0(kb[:, c, 0:1, :]),
                            start=first,
                            stop=False,
                            perf_mode=DR,
                        )
                        first = False
                    for c in range(0, C6, 2):
                        nc.tensor.matmul(
                            kp,
                            lhsT=wk_sb[:, c : c + 2, 0, m * 128 : (m + 1) * 128],
                            rhs=kb[:, c : c + 2, 1, :],
                            start=False,
                            stop=(c == C6 - 2),
                            perf_mode=DR,
                        )
                    nc.vector.tensor_scalar(
                        out=kTh[:, m, blk * BW : (blk + 1) * BW],
                        in0=kp,
                        scalar1=bkc[:, m : m + 1],
                        scalar2=1.0 / WSCALE,
                        op0=add,
                        op1=mult,
                    )

            def k_block(blk):
                for m in range(C3):
                    k_block_m(blk, m)

            def v_block_kcl(blk, kcl):
                vb = vtiles[blk]
                if True:
                    kc = blk * KCL + kcl
                    kcp, ip = kc // 2, kc % 2
                    vp = PJ.tile([128, dh], f32, tag="pj")
                    # 3-term: (vh,vl)@(wvh,wvh) per chunk + (vh_c,vh_c+1)@(wvl_c,wvl_c+1)
                    for c in range(C6):
                        nc.tensor.matmul(
                            vp,
                            lhsT=vb[:, c, :, kcl * 128 : (kcl + 1) * 128],
                            rhs=rep0(wv_sb[:, c, 0:1, :]),
                            start=(c == 0),
                            stop=False,
                            perf_mode=DR,
                        )
                    for c in range(0, C6, 2):
                        nc.tensor.matmul(
                            vp,
                            lhsT=vb[:, c : c + 2, 0, kcl * 128 : (kcl + 1) * 128],
                            rhs=wv_sb[:, c : c + 2, 1, :],
                            start=False,
                            stop=(c == C6 - 2),
                            perf_mode=DR,
                        )
                    nc.vector.scalar_tensor_tensor(
                        out=vsb[:, kcp, ip, :, 0:dk],
                        in0=vp.rearrange("p (h e) -> p h e", h=hh),
                        scalar=1.0 / WSCALE,
                        in1=bvb,
                        op0=mult,
                        op1=add,
                    )

            def v_block(blk):
                for kcl in range(KCL):
                    v_block_kcl(blk, kcl)

            # prologue: blocks 0/1 DMAs were issued with the q stream above;
            # v/k stream projections beyond k-block 0 ride head-0 slots
            k_block(0)

            # head-0 stream emission schedule: granule -> slot, chosen so
            # each k-m / v-kcl lands at least one pair before its consumer
            h0_sched = {
                0: [lambda: v_block_kcl(0, 0), lambda: v_block_kcl(0, 1),
                    lambda: k_block_m(1, 0), lambda: k_dma(2)],
                1: [lambda: v_block_kcl(0, 2), lambda: v_block_kcl(0, 3),
                    lambda: k_block_m(1, 1), lambda: k_block_m(1, 2),
                    lambda: v_dma(2)],
                2: [lambda: k_block_m(2, 0), lambda: k_block_m(2, 1),
                    lambda: q_delta(2), lambda: k_dma(3), lambda: mask_g(2)],
                3: [lambda: k_block_m(2, 2), lambda: v_block_kcl(1, 0),
                    lambda: v_block_kcl(1, 1)],
                4: [lambda: v_block_kcl(1, 2), lambda: v_block_kcl(1, 3),
                    lambda: k_block_m(3, 0), lambda: q_delta(3),
                    lambda: v_dma(3)],
                5: [lambda: k_block_m(3, 1), lambda: k_block_m(3, 2),
                    lambda: v_block_kcl(2, 0), lambda: v_block_kcl(2, 1),
                    lambda: mask_g(3)],
                6: [lambda: v_block_kcl(2, 2), lambda: v_block_kcl(2, 3),
                    lambda: nc.sync.dma_start(
                        wo_sb, t["wo"].rearrange("(c p) m -> p c m", p=128))],
                7: [lambda: v_block_kcl(3, 0), lambda: v_block_kcl(3, 1)],
            }

            # --- attention: 6 heads x 8 kc-pairs ---
            def pv(kcp, psb2, xps, h):
                for ip in range(2):
                    for nn in range(NQ):
                        nc.tensor.matmul(
                            xps[:, nn * 512 : (nn + 1) * 512],
                            lhsT=vsb[:, kcp, ip, h, :],
                            rhs=psb2[:, ip, nn * 512 : (nn + 1) * 512],
                            start=(kcp == 0 and ip == 0),
                            stop=(kcp == KCP - 1 and ip == 1),
                        )

            def pop_pv():
                # emit the oldest pending PV; when it closes a head's
                # accumulation, chain that head's 1/Z reciprocal + broadcast
                pkcp, ppsb2, pxps, ph2 = pvq.pop(0)
                pv(pkcp, ppsb2, pxps, ph2)
                if pkcp == KCP - 1:
                    rz = W2.tile([1, sq], f32, tag="rz", bufs=2)
                    nc.vector.reciprocal(rz, pxps[dk : dk + 1, :])
                    rzb = W2.tile([64, sq], f32, tag="rzb", bufs=2)
                    nc.gpsimd.partition_broadcast(rzb, rz)
                    pend.append((ph2, pxps, rzb))

            # pending PVs, emitted two pairs late (carrying across head
            # boundaries) so the mask latency and the normalize chain stay
            # off the in-order PE's critical path
            pvq = []
            pend = []
            for h in range(hh):
                hoff = (h % 2) * 64
                xps = XPP.tile([dk + 1, sq], f32, tag="xps")
                for kcp in range(KCP):
                    psb2 = W.tile([128, 2, sq], bf, tag="psb", bufs=4)
                    for ip in range(2):
                        kc = kcp * 2 + ip
                        sps = PJ.tile([128, sq], f32, tag="pj")
                        for nn in range(NQ):
                            nc.tensor.matmul(
                                sps[:, nn * 512 : (nn + 1) * 512],
                                lhsT=kTh[
                                    hoff : hoff + 64, h // 2, kc * 128 : (kc + 1) * 128
                                ],
                                rhs=qTh[
                                    hoff : hoff + 64, h // 2, nn * 512 : (nn + 1) * 512
                                ],
                                start=True,
                                stop=True,
                            )
                        nc.scalar.activation(
                            psb2[:, ip],
                            sps,
                            Exp,
                            scale=rdts[kc // KCL][:, kc % KCL : kc % KCL + 1],
                        )
                    # mask multiply over the pair, routed by (h, kcp)
                    mk = maskT[:, kcp * 2 : kcp * 2 + 2, :]
                    if kcp in POOL_KCPS:
                        nc.gpsimd.tensor_tensor(out=psb2, in0=psb2, in1=mk, op=mult)
                    else:
                        nc.vector.tensor_tensor(out=psb2, in0=psb2, in1=mk, op=mult)
                    # previous head's normalize multiply (DVE) must precede
                    # this head's first PV (same single xps slot)
                    if pend:
                        ph, pxps, przb = pend.pop(0)
                        nc.vector.tensor_tensor(
                            out=xTs[ph // 2][(ph % 2) * 64 : (ph % 2) * 64 + 64, :],
                            in0=pxps[0:dk, :],
                            in1=przb,
                            op=mult,
                        )
                    pvq.append((kcp, psb2, xps, h))
                    if len(pvq) > 2:
                        pop_pv()
                    # stream emission AFTER the attention ops, in small
                    # granules spread across head 0 (+ one slot of head 1)
                    # so no single slot of projection work starves the
                    # exp pipeline; ordering honors each granule's consumer
                    if h == 0:
                        for fn in h0_sched.get(kcp, []):
                            fn()
                    elif h == 1 and kcp == 0:
                        v_block_kcl(3, 2)
                        v_block_kcl(3, 3)
            # --- output projection (partial, this core's head dims) ---
            # c-chunks 0/1 read heads 0-3 (ready early): the first two open
            # while the last exps/masks still run, filling the PE through
            # the final PV pops; c=2 closes trail behind the last normalize
            def yproj_c(qc, yps, c, start, stop):
                for col in range(0, d, 512):
                    ncol = min(512, d - col)
                    nc.tensor.matmul(
                        yps[:, col : col + ncol],
                        lhsT=xTs[c][:, qc * 128 : (qc + 1) * 128],
                        rhs=wo_sb[:, c, col : col + ncol],
                        start=start,
                        stop=stop,
                    )

            def yproj_open(qc):
                yps = PJ.tile([128, d], f32, tag="pj")
                yproj_c(qc, yps, 0, True, False)
                yproj_c(qc, yps, 1, False, False)
                yopen.append((qc, yps))

            def yproj_close(qc, yps):
                yproj_c(qc, yps, C3 - 1, False, True)
                ysb = W2.tile([128, d], bf, tag="ysb", bufs=4)
                # both ACT and DVE are idle in the tail: split the copies
                if qc % 2 == 0:
                    nc.scalar.copy(ysb, yps)
                else:
                    nc.vector.tensor_copy(ysb, yps)
                nc.sync.dma_start(t["yp"][qc * 128 : (qc + 1) * 128, :], ysb)

            yopen = []
            yproj_open(0)
            yproj_open(1)
            # drain remaining PVs and normalizes
            while pvq:
                pop_pv()
            while pend:
                ph, pxps, przb = pend.pop(0)
                nc.vector.tensor_tensor(
                    out=xTs[ph // 2][(ph % 2) * 64 : (ph % 2) * 64 + 64, :],
                    in0=pxps[0:dk, :],
                    in1=przb,
                    op=mult,
                )
            for qc in range(2, QC):
                yproj_open(qc)
                if len(yopen) > 2:
                    yproj_close(*yopen.pop(0))
            while yopen:
                yproj_close(*yopen.pop(0))

    nc.compile()
    return nc


def _hilo(x):
    hi = x.astype(F8)
    lo = (x - hi.astype(np.float32)).astype(F8)
    return hi, lo


def _in_maps(query, key, value, mask, Wq, bq, Wk, bk, Wv, bv, Wo, Wd, bd, sq=SQ, dh=DH):
    query = np.asarray(query, np.float32)
    key = np.asarray(key, np.float32)
    value = np.asarray(value, np.float32)
    mask = np.asarray(mask)
    C6 = D // 128

    def stream_hilo(x):  # [S, D] -> [128, C6, 2, S] fp8 (hi, lo)
        xT = np.ascontiguousarray(x.T)              # [D, S]
        hi, lo = _hilo(xT)
        out = np.empty((128, C6, 2, S), F8)
        r = hi.reshape(C6, 128, S)
        out[:, :, 0] = r.transpose(1, 0, 2)
        out[:, :, 1] = lo.reshape(C6, 128, S).transpose(1, 0, 2)
        return out

    def w_hilo(w):  # [D, dh] -> [128, C6, 2, dh]
        hi, lo = _hilo(np.ascontiguousarray(w, np.float32))
        out = np.empty((128, C6, 2, w.shape[1]), F8)
        out[:, :, 0] = hi.reshape(C6, 128, -1).transpose(1, 0, 2)
        out[:, :, 1] = lo.reshape(C6, 128, -1).transpose(1, 0, 2)
        return out

    from kernel import WSCALE

    qhl = [stream_hilo(query[b]) for b in range(B)]
    khl = [stream_hilo(key[b]) for b in range(B)]
    v8 = [stream_hilo(value[b]) for b in range(B)]
    # weights pre-scaled by WSCALE before the fp8 hi/lo split so the hi part
    # stays out of e4m3's subnormal range; compensated in the bias ops
    wdf = np.ascontiguousarray(Wd, np.float32) * WSCALE  # [D, 1]
    wdh, wdl = _hilo(wdf)
    wd8 = np.zeros((128, C6, 4), F8)
    wd8[:, :, 0] = wdh.reshape(C6, 128).T
    wd8[:, :, 1] = wdh.reshape(C6, 128).T
    wd8[:, :, 2] = wdl.reshape(C6, 128).T
    wd8[:, :, 3] = wdl.reshape(C6, 128).T
    # DR2 rhs (wdl, wdl) makes delta the full 4-term product at no extra cost
    wqf = np.ascontiguousarray(Wq, np.float32) * WSCALE
    wkf = np.ascontiguousarray(Wk, np.float32) * WSCALE
    wvf = np.ascontiguousarray(Wv, np.float32) * WSCALE
    wob = np.ascontiguousarray(Wo).astype(BF16)
    bqf = np.ascontiguousarray(bq, np.float32) * WSCALE
    bkf = np.ascontiguousarray(bk, np.float32) * WSCALE
    bvf = np.ascontiguousarray(bv, np.float32)
    bdf = (np.ascontiguousarray(bd, np.float32) + 1.0) * WSCALE

    maps = []
    for c in range(NCORES):
        b, qh, hf = c // 4, (c // 2) % 2, c % 2
        qs = slice(qh * sq, (qh + 1) * sq)
        hs = slice(hf * dh, (hf + 1) * dh)
        # own q-half first in the stream so blocks [0, NBQ) are projected.
        # The same permutation is applied to the key axis everywhere
        # (khl, v8, maskT rows): attention sums over keys, so order is free
        # as long as delta, keys, values and mask rows agree.
        os_ = slice((1 - qh) * sq, (2 - qh) * sq)
        qcore = np.concatenate([qhl[b][:, :, :, qs], qhl[b][:, :, :, os_]], axis=3)
        kcore = np.concatenate([khl[b][:, :, :, qs], khl[b][:, :, :, os_]], axis=3)
        vcore = np.concatenate([v8[b][:, :, :, qs], v8[b][:, :, :, os_]], axis=3)
        mT = np.ascontiguousarray(mask[b, qs].T)  # [S(key), sq]
        mcore = np.concatenate([mT[qs], mT[os_]], axis=0)
        maps.append(
            {
                "qhl": np.ascontiguousarray(qcore),
                "khl": np.ascontiguousarray(kcore),
                "v8": np.ascontiguousarray(vcore),
                "maskT": np.ascontiguousarray(mcore).astype(BF16),
                "wq8": w_hilo(wqf[:, hs]),
                "wk8": w_hilo(wkf[:, hs]),
                "wv8": w_hilo(wvf[:, hs]),
                "wo": np.ascontiguousarray(wob[hs, :]),
                "wd8": wd8,
                "bq": np.ascontiguousarray(bqf[hs]),
                "bk": np.ascontiguousarray(bkf[hs]),
                "bv": np.ascontiguousarray(bvf[hs]),
                "bd": bdf,
            }
        )
    return maps


def kernel(query, key, value, mask, Wq, bq, Wk, bk, Wv, bv, Wo, bo, Wd, bd):
    from concourse.bass_utils import run_bass_kernel_spmd

    if "nc" not in _cache:
        _cache["nc"] = _build()
    nc = _cache["nc"]

    maps = _in_maps(query, key, value, mask, Wq, bq, Wk, bk, Wv, bv, Wo, Wd, bd)
    res = run_bass_kernel_spmd(nc, maps, core_ids=list(range(NCORES)))

    bof = np.asarray(bo, np.float32)
    y = np.empty((B, S, D), np.float32)
    for b in range(B):
        for qh in range(2):
            c0 = b * 4 + qh * 2
            y[b, qh * SQ : (qh + 1) * SQ] = (
                res.results[c0]["yp"].astype(np.float32)
                + res.results[c0 + 1]["yp"].astype(np.float32)
                + bof[None, :]
            )
    return y
